# revision 2
# baseline (speedup 1.0000x reference)
"""PillarEncoder Trainium2 kernel (v3).

Strategy (8 NeuronCores, pure SPMD, no collectives):
  core = (batch b in {0,1}) x (canvas y-region r in {0..3}, 124 rows each).
  Host bins each batch's pillars by output-canvas y-row into the owning core,
  so every core computes features for exactly the pillars that land in its
  output slice; no cross-core exchange is needed.

Per core, on device:
  1. conv(1x1)+BN folded into one K=10 fp32 matmul (bias via a ones-row of
     the feature matrix); relu+maxpool folded into max-reduce + max(.,0).
     Output `pooled` is channel-major [128=(ch x 2 slot-halves), slots/2].
  2. image bilinear sampling: 10 dma_gathers (5 top-k points x 2 y-rows,
     each row-pair fetches the x0 and x0+1 cells in one 512B descriptor),
     spread over SWDGE queues 0..3; combined with host-folded weights
     (bilinear * inside-mask / count) via DVE scalar_tensor_tensor.
  3. a channel-major pillar table V_T [128ch x (1 zero + slots)] is built in
     SBUF (pooled copied/DMA'd in; image features PE-transposed in); each
     4096-cell output chunk is then produced by a single GPSIMD ap_gather
     (free-dim gather) using the host-built inverse cell->slot map, and
     stored with one large contiguous DMA per canvas.

Host does only geometry/index/weight prep (top-k selection, projection,
bilinear weights, inverse scatter maps) and final slice assembly.
"""

import dataclasses

import numpy as np

# ---------------- constants (hardcoded from the problem spec) ----------------
B = 2
N_PTS = 32
C = 64
H_IMG, W_IMG = 96, 320
HW = H_IMG * W_IMG
X_L, Y_L = 432, 496
VX = VY = np.float32(0.16)
X_OFF = np.float32(0.16 / 2 + 0.0)
Y_OFF = np.float32(0.16 / 2 + (-39.68))
K_TOP = 5
BN_EPS = np.float32(1e-3)

N_CORES = 8
N_REG = 4                      # y-regions per batch
ROWS_REG = Y_L // N_REG        # 124
CELLS = ROWS_REG * X_L         # 53568
CHUNK = 4096
NCHUNK = 14
CELLS_PAD = CHUNK * NCHUNK     # 57344

PMAX = 3072                    # max pillars per core (padded), 24*128
HALF = PMAX // 2               # 1536
FT_COLS = PMAX * N_PTS         # 98304 conv feature columns
NG = 10                        # img gathers: 5 points x 2 y-rows
PB = PMAX // 128               # 24 slot blocks
NE = PMAX + 256                # V_T columns (col 0 = zeros)

_compiled = [None]


# =============================== device kernel ===============================
def _build():
    import concourse.tile as tile
    import concourse.bacc as bacc
    from concourse import mybir
    from concourse.masks import make_identity

    f32 = mybir.dt.float32
    i16 = mybir.dt.int16

    nc = bacc.Bacc("TRN2", target_bir_lowering=False, debug=False,
                   num_devices=N_CORES, num_swdge_queues=4)

    featsT = nc.dram_tensor("featsT", [10, FT_COLS], f32, kind="ExternalInput")
    lhsT_d = nc.dram_tensor("lhsT", [10, 256], f32, kind="ExternalInput")
    img_d = nc.dram_tensor("img", [HW + 2, C], f32, kind="ExternalInput")
    gidx_d = nc.dram_tensor("gidx", [128, NG * (PMAX // 16)], i16,
                            kind="ExternalInput")
    gw_d = nc.dram_tensor("gw", [128, NG * PB * 2], f32, kind="ExternalInput")
    cidx_d = nc.dram_tensor("cidx", [128, NCHUNK * (CHUNK // 16)], i16,
                            kind="ExternalInput")
    out_l = nc.dram_tensor("out_l", [C, CELLS_PAD], f32, kind="ExternalOutput")
    out_i = nc.dram_tensor("out_i", [C, CELLS_PAD], f32, kind="ExternalOutput")

    # paired-row gather view: row stride 64 elems, 128-elem (2-cell) reads
    img_pair_ap = dataclasses.replace(img_d[:, :], ap=[[C, HW], [1, 2 * C]])

    with tile.TileContext(nc) as tc:
        with (
            tc.tile_pool(name="const", bufs=1) as cpool,
            tc.tile_pool(name="ft", bufs=3) as ftpool,
            tc.tile_pool(name="gimg", bufs=4) as gpool,
            tc.tile_pool(name="acc", bufs=1) as apool,
            tc.tile_pool(name="fout", bufs=4) as fpool,
            tc.tile_pool(name="psA", bufs=3, space="PSUM") as psA,
            tc.tile_pool(name="psV", bufs=2, space="PSUM") as psV,
        ):
            # ---- constants / small loads ----
            ident = cpool.tile([128, 128], f32)
            make_identity(nc, ident[:])
            lhsT = cpool.tile([10, 256], f32)
            nc.sync.dma_start(lhsT[:], lhsT_d[:])
            gidx = cpool.tile([128, NG * (PMAX // 16)], i16)
            nc.sync.dma_start(gidx[:], gidx_d[:])
            gw = cpool.tile([128, NG * PB * 2], f32)
            nc.sync.dma_start(gw[:], gw_d[:])
            cidx = cpool.tile([128, NCHUNK * (CHUNK // 16)], i16)
            nc.sync.dma_start(cidx[:], cidx_d[:])

            # ---- phase C: conv+BN matmul, maxpool ----
            pooled = apool.tile([128, HALF], f32)
            for u in range(FT_COLS // 4096):
                ft = ftpool.tile([10, 4096], f32)
                nc.sync.dma_start(ft[:], featsT[:, u * 4096:(u + 1) * 4096])
                for tt in range(2):
                    t = 2 * u + tt
                    h = psA.tile([128, 1024], f32, space="PSUM")
                    for q in range(2):
                        base = tt * 2048 + q * 1024
                        nc.tensor.matmul(h[:, q * 512:(q + 1) * 512],
                                         lhsT[:, 0:128],
                                         ft[:, base:base + 512],
                                         start=True, stop=False)
                        nc.tensor.matmul(h[:, q * 512:(q + 1) * 512],
                                         lhsT[:, 128:256],
                                         ft[:, base + 512:base + 1024],
                                         start=False, stop=True)
                    nc.vector.reduce_max(
                        pooled[:, 32 * t:32 * (t + 1)],
                        h[:].rearrange("p (i n) -> p i n", n=N_PTS),
                        axis=mybir.AxisListType.X)
            nc.vector.tensor_scalar_max(pooled[:], pooled[:], 0.0)

            # ---- phase E: bilinear image features (slot-major iacc) ----
            # iacc block u lives at cols [64+64u, 128+64u); col 0:64 zero pad
            # so each PE transpose reads an aligned [128,128] window.
            iacc = apool.tile([128, 64 * (PB + 1)], f32)
            nc.vector.memset(iacc[:, 0:64], 0.0)
            for g in range(NG):
                gt = gpool.tile([128, PB, 2 * C], f32, tag="g")
                nc.gpsimd.dma_gather(
                    gt[:], img_pair_ap,
                    gidx[:, g * (PMAX // 16):(g + 1) * (PMAX // 16)],
                    PMAX, PMAX, 2 * C, elem_step=C,
                    single_packet=False, queue_num=g % 4)
                for u in range(PB):
                    dst = iacc[:, 64 + 64 * u:128 + 64 * u]
                    for hf in range(2):
                        w = gw[:, g * 2 * PB + 2 * u + hf:
                               g * 2 * PB + 2 * u + hf + 1]
                        src = gt[:, u, hf * C:(hf + 1) * C]
                        if g == 0 and hf == 0:
                            nc.vector.tensor_scalar(
                                dst, src, w, None, op0=mybir.AluOpType.mult)
                        else:
                            nc.vector.scalar_tensor_tensor(
                                dst, src, w, dst,
                                op0=mybir.AluOpType.mult,
                                op1=mybir.AluOpType.add)

            # ---- phase V_T: channel-major pillar table ----
            vt = apool.tile([128, NE], f32)
            nc.vector.memset(vt[:, 0:1], 0.0)
            nc.vector.memset(vt[:, 1 + PMAX:], 0.0)
            nc.vector.tensor_copy(vt[0:64, 1:1 + HALF], pooled[0:64, :])
            nc.sync.dma_start(vt[0:64, 1 + HALF:1 + PMAX], pooled[64:128, :])
            for u in range(PB):
                pv = psV.tile([128, 128], f32, space="PSUM")
                nc.tensor.transpose(pv[:], iacc[:, 64 * u:64 * u + 128],
                                    ident[:])
                nc.scalar.copy(vt[64:128, 1 + 128 * u:1 + 128 * (u + 1)],
                               pv[64:128, :])

            # ---- phase F: per-cell free-dim gather, store ----
            import os as _os
            _probe = _os.environ.get("KPROBE", "")
            for j in range(NCHUNK):
                fo = fpool.tile([128, CHUNK], f32)
                if _probe == "memset":
                    nc.vector.memset(fo[:], 0.125)
                else:
                    nc.gpsimd.ap_gather(
                        fo[:].rearrange("p (n d) -> p n d", d=1),
                        vt[:].rearrange("p (n d) -> p n d", d=1),
                        cidx[:, j * (CHUNK // 16):(j + 1) * (CHUNK // 16)],
                        channels=128, num_elems=NE, d=1, num_idxs=CHUNK)
                nc.sync.dma_start(out_l[:, CHUNK * j:CHUNK * (j + 1)],
                                  fo[0:64, :])
                nc.scalar.dma_start(out_i[:, CHUNK * j:CHUNK * (j + 1)],
                                    fo[64:128, :])

    nc.compile()
    return nc


def _get_nc():
    if _compiled[0] is None:
        _compiled[0] = _build()
    return _compiled[0]


# ================================ host prep =================================
def _slot_cols(slots):
    """featsT column base for each slot (conv psum-tile layout)."""
    s = np.asarray(slots)
    h = (s >= HALF).astype(np.int64)
    sl = s - h * HALF
    t = sl // 16
    i = sl % 16
    return 1024 * t + 512 * h + 32 * i


def _host_prep(pillars, coors_batch, npoints_per_pillar, batched_image_map,
               image_shapes, proj_mats, conv_w, bn_gamma, bn_beta, bn_mean,
               bn_var):
    pillars = np.asarray(pillars, np.float32)
    coors = np.asarray(coors_batch, np.int64)
    npoints = np.asarray(npoints_per_pillar, np.int64)
    img = np.asarray(batched_image_map, np.float32)
    ish = np.asarray(image_shapes, np.int64)
    projm = np.asarray(proj_mats, np.float32)
    conv_w = np.asarray(conv_w, np.float32)
    bn_gamma = np.asarray(bn_gamma, np.float32)
    bn_beta = np.asarray(bn_beta, np.float32)
    bn_mean = np.asarray(bn_mean, np.float32)
    bn_var = np.asarray(bn_var, np.float32)

    xyz = pillars[:, :, :3]
    npf = npoints.astype(np.float32)
    mean = xyz.sum(axis=1) / npf[:, None]
    offset = xyz - mean[:, None, :]
    mask = np.arange(N_PTS)[None, :] < npoints[:, None]
    cx = coors[:, 1].astype(np.float32) * VX + X_OFF
    cy = coors[:, 2].astype(np.float32) * VY + Y_OFF
    x_off = pillars[:, :, 0] - cx[:, None]
    y_off = pillars[:, :, 1] - cy[:, None]
    feats9 = np.stack([x_off, y_off, pillars[:, :, 2], pillars[:, :, 3],
                       offset[:, :, 0], offset[:, :, 1], offset[:, :, 2],
                       x_off, y_off], axis=-1)
    feats9 = feats9 * mask[:, :, None].astype(np.float32)

    # top-K closest valid points (stable ties like lax.top_k)
    dists = np.where(mask, (offset * offset).sum(-1), np.float32(1e6))
    tidx = np.argsort(dists, axis=1, kind="stable")[:, :K_TOP]
    tmask = np.take_along_axis(mask, tidx, axis=1)
    txyz = np.take_along_axis(xyz, tidx[..., None], axis=1)

    b_idx = coors[:, 0]
    Mb = projm[b_idx]                                    # (P,3,4)
    hom = np.concatenate([txyz, np.ones_like(txyz[..., :1])], axis=-1)
    proj = np.einsum("pij,pkj->pki", Mb, hom).astype(np.float32)
    u = proj[..., 0] / proj[..., 2]
    v = proj[..., 1] / proj[..., 2]
    shf = ish[b_idx].astype(np.float32)                  # (P,2) H_img,W_img
    u_s = u * (np.float32(W_IMG) / shf[:, 1:2])
    v_s = v * (np.float32(H_IMG) / shf[:, 0:1])
    inside = (tmask & (u_s >= 0) & (u_s <= W_IMG - 1)
              & (v_s >= 0) & (v_s <= H_IMG - 1))
    u_c = np.clip(u_s, np.float32(0.0), np.float32(W_IMG - 1))
    v_c = np.clip(v_s, np.float32(0.0), np.float32(H_IMG - 1))
    x0 = np.floor(u_c).astype(np.int32)
    y0 = np.floor(v_c).astype(np.int32)
    y1 = np.minimum(y0 + 1, H_IMG - 1)
    wx = u_c - x0.astype(np.float32)
    wy = v_c - y0.astype(np.float32)
    w_in = inside.astype(np.float64)
    cnt = np.maximum(w_in.sum(axis=1), 1.0)
    wk = w_in / cnt[:, None]                             # (P,K)
    wx64, wy64 = wx.astype(np.float64), wy.astype(np.float64)
    # per (point, y-row h, x-half): folded bilinear weights
    cw = (np.stack([(1 - wx64) * (1 - wy64), wx64 * (1 - wy64),
                    (1 - wx64) * wy64, wx64 * wy64], axis=-1)
          * wk[..., None]).astype(np.float32)            # (P,K,4): 00,01,10,11
    rows_pair = np.stack([y0 * W_IMG + x0, y1 * W_IMG + x0], -1)  # (P,K,2)

    # folded conv+BN
    scale = bn_gamma / np.sqrt(bn_var + BN_EPS)
    Wp = conv_w * scale[:, None]                         # (64,9)
    bp = bn_beta - bn_mean * scale
    lhsT = np.zeros((10, 256), np.float32)
    lhsT[0:9, 0:64] = Wp.T
    lhsT[9, 0:64] = bp
    lhsT[0:9, 192:256] = Wp.T
    lhsT[9, 192:256] = bp

    xc = coors[:, 1].astype(np.int64)
    yc = coors[:, 2].astype(np.int64)

    in_maps = []
    for core in range(N_CORES):
        b, r = divmod(core, N_REG)
        sel = np.where((b_idx == b) & (yc >= ROWS_REG * r)
                       & (yc < ROWS_REG * (r + 1)))[0]
        lin = (yc[sel] - ROWS_REG * r) * X_L + xc[sel]
        order = np.argsort(lin, kind="stable")
        sel = sel[order]
        lin = lin[order]
        n_p = len(sel)
        assert n_p <= PMAX, f"core {core}: {n_p} pillars > PMAX {PMAX}"

        ftT = np.zeros((10, FT_COLS), np.float32)
        ftT[9, :] = 1.0
        colb = _slot_cols(np.arange(n_p))
        cols = (colb[:, None] + np.arange(N_PTS)[None, :]).ravel()
        ftT[0:9].reshape(9, FT_COLS)[:, cols] = (
            feats9[sel].transpose(2, 0, 1).reshape(9, n_p * N_PTS))

        gi = np.zeros((NG, PMAX), np.int16)
        gwv = np.zeros((NG, PB * 2, 128), np.float32)
        for k in range(K_TOP):
            for hf in range(2):
                g = k * 2 + hf
                gi[g, :n_p] = rows_pair[sel, k, hf].astype(np.int16)
                wlo = np.zeros(PMAX, np.float32)
                whi = np.zeros(PMAX, np.float32)
                wlo[:n_p] = cw[sel, k, 2 * hf]        # x0 cell
                whi[:n_p] = cw[sel, k, 2 * hf + 1]    # x1 cell
                gwv[g, 0::2, :] = wlo.reshape(PB, 128)
                gwv[g, 1::2, :] = whi.reshape(PB, 128)
        gidx = (gi.reshape(NG, PMAX // 16, 16).transpose(2, 0, 1)
                .reshape(16, NG * (PMAX // 16)))
        gidx = np.tile(gidx, (8, 1))
        gw = gwv.reshape(NG * PB * 2, 128).T             # (128, NG*PB*2)

        cvals = np.zeros(CELLS_PAD, np.int16)
        cvals[lin] = (1 + np.arange(n_p)).astype(np.int16)
        cidx = (cvals.reshape(NCHUNK, CHUNK // 16, 16).transpose(2, 0, 1)
                .reshape(16, NCHUNK * (CHUNK // 16)))
        cidx = np.tile(cidx, (8, 1))

        imgt = np.zeros((HW + 2, C), np.float32)
        imgt[:HW] = img[b].transpose(1, 2, 0).reshape(HW, C)

        in_maps.append({
            "featsT": ftT,
            "lhsT": lhsT,
            "img": imgt,
            "gidx": np.ascontiguousarray(gidx),
            "gw": np.ascontiguousarray(gw),
            "cidx": np.ascontiguousarray(cidx),
        })
    return in_maps


def _assemble(results):
    canvas_l = np.zeros((B, C, Y_L, X_L), np.float32)
    canvas_i = np.zeros((B, C, Y_L, X_L), np.float32)
    for core in range(N_CORES):
        b, r = divmod(core, N_REG)
        ysl = slice(ROWS_REG * r, ROWS_REG * (r + 1))
        canvas_l[b, :, ysl, :] = (
            results[core]["out_l"][:, :CELLS].reshape(C, ROWS_REG, X_L))
        canvas_i[b, :, ysl, :] = (
            results[core]["out_i"][:, :CELLS].reshape(C, ROWS_REG, X_L))
    return canvas_l, canvas_i


def kernel(**inputs):
    from concourse.bass_utils import run_bass_kernel_spmd

    nc = _get_nc()
    in_maps = _host_prep(**inputs)
    res = run_bass_kernel_spmd(nc, in_maps, list(range(N_CORES)))
    return _assemble(res.results)



# revision 9
# speedup vs baseline: 6.1214x; 6.1214x over previous
"""PillarEncoder Trainium2 kernel (v4).

Strategy (8 NeuronCores, pure SPMD, no collectives):
  core = (batch b in {0,1}) x (canvas y-region r in {0..3}, 124 rows each).
  Host bins each batch's pillars by output-canvas y-row into the owning core,
  so every core computes features for exactly the pillars that land in its
  output slice; no cross-core exchange is needed.

Per core, on device (all heavy data paths redesigned vs v3):
  1. conv(1x1)+BN folded into one K=10 matmul in bf16 (fp32 matmul costs
     4 cyc/row on TRN2 PE; bf16 costs 1). relu folded into the later
     Activation-engine copies; maxpool = DVE reduce_max on PSUM.
  2. image bilinear sampling: 10 dma_gathers (5 top-k points x 2 y-rows,
     each fetching the x0/x1 cell pair in one 512B descriptor). Bilinear
     weights are applied with broadcast-AP tensor_tensor ops (w varies
     along the free dim), ~3 DVE ops per gather instead of 48
     scalar_tensor_tensor ops.
  3. output: a slot-major pillar table V [slot(part) x 128ch] in bf16
     (cols 0:64 pooled, 64:128 image feat). The BEV canvases live in DRAM
     as one [4 windows x (13392+1 pad) rows, 128ch] bf16 tensor: big DMAs
     zero-fill it at kernel start (overlapped under compute), then 4 SWDGE
     dma_scatter_add calls (one per 13392-row window so cell indices fit
     int16) add the ~2.5k pillar rows (256B descriptors) onto the zeroed
     canvas. Pad slots carry exactly-zero V rows and target each window's
     private pad row, so their adds are no-ops. This replaces v3's 14
     GPSIMD ap_gathers (~100us each) entirely.

Host does geometry/index/weight prep (top-k selection, projection, bilinear
weights, scatter cell indices) and final slice assembly (split + transpose).
"""

import dataclasses

import numpy as np

# ---------------- constants (hardcoded from the problem spec) ----------------
B = 2
N_PTS = 32
C = 64
H_IMG, W_IMG = 96, 320
HW = H_IMG * W_IMG
X_L, Y_L = 432, 496
VX = VY = np.float32(0.16)
X_OFF = np.float32(0.16 / 2 + 0.0)
Y_OFF = np.float32(0.16 / 2 + (-39.68))
K_TOP = 5
BN_EPS = np.float32(1e-3)

N_CORES = 8
N_REG = 4                      # y-regions per batch
ROWS_REG = Y_L // N_REG        # 124
CELLS = ROWS_REG * X_L         # 53568
PMAX = 3072                    # max pillars per core (padded), 24*128
HALF = PMAX // 2               # 1536
FT_COLS = PMAX * N_PTS         # 98304 conv feature columns
NG = 10                        # img gathers: 5 points x 2 y-rows
PB = PMAX // 128               # 24 slot blocks

NWIN = 4                       # scatter windows (int16 cell-index range)
WROWS = CELLS // NWIN          # 13392 canvas rows per window
WSLOTS = PMAX // NWIN          # 768 slots per window
WBLK = PB // NWIN              # 6 vt blocks per window
ROWS_PAD = WROWS + 1           # +1 private pad row per window
OUT_ROWS = NWIN * ROWS_PAD     # 53572

_compiled = [None]


# =============================== device kernel ===============================
def _build():
    import concourse.tile as tile
    import concourse.bacc as bacc
    from concourse import mybir
    from concourse.masks import make_identity

    f32 = mybir.dt.float32
    bf16 = mybir.dt.bfloat16
    i16 = mybir.dt.int16

    nc = bacc.Bacc("TRN2", target_bir_lowering=False, debug=False,
                   num_devices=N_CORES, num_swdge_queues=4)

    featsT = nc.dram_tensor("featsT", [10, FT_COLS], bf16, kind="ExternalInput")
    lhsT_d = nc.dram_tensor("lhsT", [10, 256], bf16, kind="ExternalInput")
    img_d = nc.dram_tensor("img", [HW + 2, C], f32, kind="ExternalInput")
    gidx_d = nc.dram_tensor("gidx", [128, NG * (PMAX // 16)], i16,
                            kind="ExternalInput")
    gw2_d = nc.dram_tensor("gw2", [128, NG * PB * 2], f32,
                           kind="ExternalInput")
    widx_d = nc.dram_tensor("widx", [128, NWIN * (WSLOTS // 16)], i16,
                            kind="ExternalInput")
    out_li = nc.dram_tensor("out_li", [OUT_ROWS, 128], bf16,
                            kind="ExternalOutput")

    # paired-row gather view: row stride 64 elems, 128-elem (2-cell) reads
    img_pair_ap = dataclasses.replace(img_d[:, :], ap=[[C, HW], [1, 2 * C]])

    with tile.TileContext(nc) as tc:
        with (
            tc.tile_pool(name="const", bufs=1) as cpool,
            tc.tile_pool(name="zt", bufs=1) as zpool,
            tc.tile_pool(name="ft", bufs=3) as ftpool,
            tc.tile_pool(name="gimg", bufs=3) as gpool,
            tc.tile_pool(name="tmp", bufs=2) as tpool,
            tc.tile_pool(name="acc", bufs=1) as apool,
            tc.tile_pool(name="psA", bufs=3, space="PSUM") as psA,
            tc.tile_pool(name="psT", bufs=2, space="PSUM") as psT,
        ):
            # ---- zero-fill the output canvas first (overlaps all compute) --
            zt = zpool.tile([128, ROWS_PAD], bf16)
            nc.vector.memset(zt[:], 0.0)
            for i in range(NWIN):
                dst = dataclasses.replace(
                    out_li[:, :], ap=[[ROWS_PAD, 128], [1, ROWS_PAD]],
                    offset=i * 128 * ROWS_PAD)
                eng = nc.sync if i % 2 == 0 else nc.scalar
                eng.dma_start(dst, zt[:])

            # ---- constants / small loads ----
            ident = cpool.tile([128, 128], bf16)
            make_identity(nc, ident[:])
            lhsT = cpool.tile([10, 256], bf16)
            nc.sync.dma_start(lhsT[:], lhsT_d[:])
            gidx = cpool.tile([128, NG * (PMAX // 16)], i16)
            nc.sync.dma_start(gidx[:], gidx_d[:])
            gw2 = cpool.tile([128, NG * PB * 2], f32)
            nc.sync.dma_start(gw2[:], gw2_d[:])
            widx = cpool.tile([128, NWIN * (WSLOTS // 16)], i16)
            nc.sync.dma_start(widx[:], widx_d[:])

            # slot-major pillar table: [slot%128, slot//128, 128ch]
            vt = apool.tile([128, PB, 128], bf16)
            pooled = apool.tile([128, HALF], bf16)

            # ---- phase C: conv+BN matmul (bf16), maxpool ----
            for u in range(FT_COLS // 4096):
                ft = ftpool.tile([10, 4096], bf16)
                nc.sync.dma_start(ft[:], featsT[:, u * 4096:(u + 1) * 4096])
                for tt in range(2):
                    t = 2 * u + tt
                    h = psA.tile([128, 1024], f32, space="PSUM")
                    for q in range(2):
                        base = tt * 2048 + q * 1024
                        nc.tensor.matmul(h[:, q * 512:(q + 1) * 512],
                                         lhsT[:, 0:128],
                                         ft[:, base:base + 512],
                                         start=True, stop=False)
                        nc.tensor.matmul(h[:, q * 512:(q + 1) * 512],
                                         lhsT[:, 128:256],
                                         ft[:, base + 512:base + 1024],
                                         start=False, stop=True)
                    nc.vector.reduce_max(
                        pooled[:, 32 * t:32 * (t + 1)],
                        h[:].rearrange("p (i n) -> p i n", n=N_PTS),
                        axis=mybir.AxisListType.X)

            # ---- transpose pooled into slot-major vt (relu via Act copy) ---
            # pooled col j = slot j (rows 0:64) / slot 1536+j (rows 64:128)
            for w in range(12):
                pv = psT.tile([128, 128], bf16, space="PSUM")
                nc.tensor.transpose(pv[:], pooled[:, 128 * w:128 * (w + 1)],
                                    ident[:])
                nc.scalar.activation(vt[:, w, 0:64], pv[:, 0:64],
                                     func=mybir.ActivationFunctionType.Relu)
                nc.scalar.activation(vt[:, 12 + w, 0:64], pv[:, 64:128],
                                     func=mybir.ActivationFunctionType.Relu)

            # ---- phase E: bilinear image features (slot-major) ----
            acc = apool.tile([128, PB, C], f32)
            qmap = [0, 1, 2, 3, 0, 1, 2, 3, 0, 1]
            for g in range(NG):
                gt = gpool.tile([128, PB, 2 * C], f32, tag="g")
                nc.gpsimd.dma_gather(
                    gt[:], img_pair_ap,
                    gidx[:, g * (PMAX // 16):(g + 1) * (PMAX // 16)],
                    PMAX, PMAX, 2 * C, elem_step=C,
                    single_packet=False, queue_num=qmap[g])
                tmp = tpool.tile([128, PB, 2, C], f32, tag="t")
                wap = (gw2[:, g * 2 * PB:(g + 1) * 2 * PB]
                       .broadcast_to([128, 2 * PB, C]))
                nc.vector.tensor_tensor(
                    tmp[:].rearrange("p u h c -> p (u h) c"),
                    gt[:].rearrange("p u c2 -> p (u c2)")
                         .rearrange("p (f c) -> p f c", c=C),
                    wap, op=mybir.AluOpType.mult)
                if g == 0:
                    nc.vector.tensor_tensor(
                        acc[:], tmp[:, :, 0, :], tmp[:, :, 1, :],
                        op=mybir.AluOpType.add)
                else:
                    nc.vector.tensor_tensor(
                        acc[:], acc[:], tmp[:, :, 0, :],
                        op=mybir.AluOpType.add)
                    dst = (vt[:, :, C:2 * C] if g == NG - 1 else acc[:])
                    nc.vector.tensor_tensor(
                        dst, acc[:], tmp[:, :, 1, :],
                        op=mybir.AluOpType.add)

            # ---- phase F: sparse scatter-add of pillar rows onto the zeroed
            # canvas. One call per 13392-row window keeps indices in int16.
            # SWDGE sem lanes are assigned round-robin (mod 8) over Pool DMA
            # instructions and each lane is locked to one queue: gathers are
            # lanes 0..7,0,1 (queues qmap), scatters lanes 2,3,4,5 must reuse
            # those lanes' queues -> [2, 3, 0, 1].
            for c in range(NWIN):
                dstw = dataclasses.replace(
                    out_li[:, :], ap=[[128, ROWS_PAD], [1, 128]],
                    offset=c * ROWS_PAD * 128)
                nc.gpsimd.dma_scatter_add(
                    dstw, vt[:, WBLK * c:WBLK * (c + 1), :],
                    widx[:, (WSLOTS // 16) * c:(WSLOTS // 16) * (c + 1)],
                    WSLOTS, WSLOTS, 128, elem_step=128,
                    single_packet=False, queue_num=[2, 3, 0, 1][c])

    nc.compile()
    return nc


def _get_nc():
    if _compiled[0] is None:
        _compiled[0] = _build()
    return _compiled[0]


# ================================ host prep =================================
def _slot_cols(slots):
    """featsT column base for each slot (conv psum-tile layout)."""
    s = np.asarray(slots)
    h = (s >= HALF).astype(np.int64)
    sl = s - h * HALF
    t = sl // 16
    i = sl % 16
    return 1024 * t + 512 * h + 32 * i


def _wrap16(a):
    """[N] -> [128, N//16] wrapped-16 index layout (replicated x8)."""
    n = a.shape[0]
    w = a.reshape(n // 16, 16).T
    return np.tile(w, (8, 1))


def _host_prep(pillars, coors_batch, npoints_per_pillar, batched_image_map,
               image_shapes, proj_mats, conv_w, bn_gamma, bn_beta, bn_mean,
               bn_var):
    from concourse import mybir
    np_bf16 = mybir.dt.np(mybir.dt.bfloat16)

    pillars = np.asarray(pillars, np.float32)
    coors = np.asarray(coors_batch, np.int64)
    npoints = np.asarray(npoints_per_pillar, np.int64)
    img = np.asarray(batched_image_map, np.float32)
    ish = np.asarray(image_shapes, np.int64)
    projm = np.asarray(proj_mats, np.float32)
    conv_w = np.asarray(conv_w, np.float32)
    bn_gamma = np.asarray(bn_gamma, np.float32)
    bn_beta = np.asarray(bn_beta, np.float32)
    bn_mean = np.asarray(bn_mean, np.float32)
    bn_var = np.asarray(bn_var, np.float32)

    xyz = pillars[:, :, :3]
    npf = npoints.astype(np.float32)
    mean = xyz.sum(axis=1) / npf[:, None]
    offset = xyz - mean[:, None, :]
    mask = np.arange(N_PTS)[None, :] < npoints[:, None]
    cx = coors[:, 1].astype(np.float32) * VX + X_OFF
    cy = coors[:, 2].astype(np.float32) * VY + Y_OFF
    x_off = pillars[:, :, 0] - cx[:, None]
    y_off = pillars[:, :, 1] - cy[:, None]
    feats9 = np.stack([x_off, y_off, pillars[:, :, 2], pillars[:, :, 3],
                       offset[:, :, 0], offset[:, :, 1], offset[:, :, 2],
                       x_off, y_off], axis=-1)
    feats9 = feats9 * mask[:, :, None].astype(np.float32)

    # top-K closest valid points (stable ties like lax.top_k)
    dists = np.where(mask, (offset * offset).sum(-1), np.float32(1e6))
    tidx = np.argsort(dists, axis=1, kind="stable")[:, :K_TOP]
    tmask = np.take_along_axis(mask, tidx, axis=1)
    txyz = np.take_along_axis(xyz, tidx[..., None], axis=1)

    b_idx = coors[:, 0]
    Mb = projm[b_idx]                                    # (P,3,4)
    hom = np.concatenate([txyz, np.ones_like(txyz[..., :1])], axis=-1)
    proj = np.einsum("pij,pkj->pki", Mb, hom).astype(np.float32)
    u = proj[..., 0] / proj[..., 2]
    v = proj[..., 1] / proj[..., 2]
    shf = ish[b_idx].astype(np.float32)                  # (P,2) H_img,W_img
    u_s = u * (np.float32(W_IMG) / shf[:, 1:2])
    v_s = v * (np.float32(H_IMG) / shf[:, 0:1])
    inside = (tmask & (u_s >= 0) & (u_s <= W_IMG - 1)
              & (v_s >= 0) & (v_s <= H_IMG - 1))
    u_c = np.clip(u_s, np.float32(0.0), np.float32(W_IMG - 1))
    v_c = np.clip(v_s, np.float32(0.0), np.float32(H_IMG - 1))
    x0 = np.floor(u_c).astype(np.int32)
    y0 = np.floor(v_c).astype(np.int32)
    y1 = np.minimum(y0 + 1, H_IMG - 1)
    wx = u_c - x0.astype(np.float32)
    wy = v_c - y0.astype(np.float32)
    w_in = inside.astype(np.float64)
    cnt = np.maximum(w_in.sum(axis=1), 1.0)
    wk = w_in / cnt[:, None]                             # (P,K)
    wx64, wy64 = wx.astype(np.float64), wy.astype(np.float64)
    # per (point, y-row h, x-half): folded bilinear weights
    cw = (np.stack([(1 - wx64) * (1 - wy64), wx64 * (1 - wy64),
                    (1 - wx64) * wy64, wx64 * wy64], axis=-1)
          * wk[..., None]).astype(np.float32)            # (P,K,4): 00,01,10,11
    rows_pair = np.stack([y0 * W_IMG + x0, y1 * W_IMG + x0], -1)  # (P,K,2)

    # folded conv+BN
    scale = bn_gamma / np.sqrt(bn_var + BN_EPS)
    Wp = conv_w * scale[:, None]                         # (64,9)
    bp = bn_beta - bn_mean * scale
    lhsT = np.zeros((10, 256), np.float32)
    lhsT[0:9, 0:64] = Wp.T
    lhsT[9, 0:64] = bp
    lhsT[0:9, 192:256] = Wp.T
    lhsT[9, 192:256] = bp
    lhsT = lhsT.astype(np_bf16)

    xc = coors[:, 1].astype(np.int64)
    yc = coors[:, 2].astype(np.int64)

    in_maps = []
    for core in range(N_CORES):
        b, r = divmod(core, N_REG)
        sel = np.where((b_idx == b) & (yc >= ROWS_REG * r)
                       & (yc < ROWS_REG * (r + 1)))[0]
        lin = (yc[sel] - ROWS_REG * r) * X_L + xc[sel]
        order = np.argsort(lin, kind="stable")
        sel = sel[order]
        lin = lin[order]
        n_p = len(sel)
        assert n_p <= PMAX, f"core {core}: {n_p} pillars > PMAX {PMAX}"

        # window-packed slot assignment: window c gets slots
        # [WSLOTS*c, WSLOTS*(c+1)); pads keep zero V rows and target the
        # window's private pad row WROWS.
        win = lin // WROWS                               # 0..3, sorted
        rel = (lin - win * WROWS).astype(np.int16)
        slot = np.empty(n_p, np.int64)
        widx = np.full(PMAX, WROWS, np.int16)
        for c in range(NWIN):
            m = np.nonzero(win == c)[0]
            assert len(m) <= WSLOTS, \
                f"core {core} window {c}: {len(m)} pillars > {WSLOTS}"
            slot[m] = WSLOTS * c + np.arange(len(m))
            widx[WSLOTS * c:WSLOTS * c + len(m)] = rel[m]

        ftT = np.zeros((10, FT_COLS), np_bf16)
        colb = _slot_cols(slot)
        cols = (colb[:, None] + np.arange(N_PTS)[None, :]).ravel()
        ftT[9, cols] = 1.0
        ftT[0:9].reshape(9, FT_COLS)[:, cols] = (
            feats9[sel].transpose(2, 0, 1).reshape(9, n_p * N_PTS)
            .astype(np_bf16))

        # gather indices (pads fetch row 0, weight 0) + folded weights
        gi = np.zeros((NG, PMAX), np.int16)
        gwv = np.zeros((NG, PB * 2, 128), np.float32)
        for k in range(K_TOP):
            for hf in range(2):
                g = k * 2 + hf
                gi[g, slot] = rows_pair[sel, k, hf].astype(np.int16)
                wlo = np.zeros(PMAX, np.float32)
                whi = np.zeros(PMAX, np.float32)
                wlo[slot] = cw[sel, k, 2 * hf]        # x0 cell
                whi[slot] = cw[sel, k, 2 * hf + 1]    # x1 cell
                gwv[g, 0::2, :] = wlo.reshape(PB, 128)
                gwv[g, 1::2, :] = whi.reshape(PB, 128)
        gidx = _wrap16(gi.reshape(NG * PMAX)).reshape(
            128, NG, PMAX // 16).reshape(128, NG * (PMAX // 16))
        gw2 = gwv.reshape(NG * PB * 2, 128).T            # (128, NG*PB*2)

        imgt = np.zeros((HW + 2, C), np.float32)
        imgt[:HW] = img[b].transpose(1, 2, 0).reshape(HW, C)

        in_maps.append({
            "featsT": ftT,
            "lhsT": lhsT,
            "img": imgt,
            "gidx": np.ascontiguousarray(gidx),
            "gw2": np.ascontiguousarray(gw2),
            "widx": np.ascontiguousarray(_wrap16(widx)),
        })
    return in_maps


def _assemble(results):
    canvas_l = np.zeros((B, C, Y_L, X_L), np.float32)
    canvas_i = np.zeros((B, C, Y_L, X_L), np.float32)
    for core in range(N_CORES):
        b, r = divmod(core, N_REG)
        ysl = slice(ROWS_REG * r, ROWS_REG * (r + 1))
        arr = (results[core]["out_li"].reshape(NWIN, ROWS_PAD, 128)[:, :WROWS]
               .reshape(CELLS, 128).astype(np.float32))
        canvas_l[b, :, ysl, :] = arr[:, 0:C].T.reshape(C, ROWS_REG, X_L)
        canvas_i[b, :, ysl, :] = arr[:, C:2 * C].T.reshape(C, ROWS_REG, X_L)
    return canvas_l, canvas_i


def kernel(**inputs):
    from concourse.bass_utils import run_bass_kernel_spmd

    nc = _get_nc()
    in_maps = _host_prep(**inputs)
    res = run_bass_kernel_spmd(nc, in_maps, list(range(N_CORES)))
    return _assemble(res.results)


# revision 16
# speedup vs baseline: 6.7519x; 1.1030x over previous
"""PillarEncoder Trainium2 kernel (v4).

Strategy (8 NeuronCores, pure SPMD, no collectives):
  core = (batch b in {0,1}) x (canvas y-region r in {0..3}, 124 rows each).
  Host bins each batch's pillars by output-canvas y-row into the owning core,
  so every core computes features for exactly the pillars that land in its
  output slice; no cross-core exchange is needed.

Per core, on device (all heavy data paths redesigned vs v3):
  1. conv(1x1)+BN folded into one K=10 matmul in bf16 (fp32 matmul costs
     4 cyc/row on TRN2 PE; bf16 costs 1). relu folded into the later
     Activation-engine copies; maxpool = DVE reduce_max on PSUM.
  2. image bilinear sampling: 10 dma_gathers (5 top-k points x 2 y-rows,
     each fetching the x0/x1 cell pair in one 512B descriptor). Bilinear
     weights are applied with broadcast-AP tensor_tensor ops (w varies
     along the free dim), ~3 DVE ops per gather instead of 48
     scalar_tensor_tensor ops.
  3. output: a slot-major pillar table V [slot(part) x 128ch] in bf16
     (cols 0:64 pooled, 64:128 image feat). The BEV canvases live in DRAM
     as one [4 windows x (13392+1 pad) rows, 128ch] bf16 tensor: big DMAs
     zero-fill it at kernel start (overlapped under compute), then 4 SWDGE
     dma_scatter_add calls (one per 13392-row window so cell indices fit
     int16) add the ~2.5k pillar rows (256B descriptors) onto the zeroed
     canvas. Pad slots carry exactly-zero V rows and target each window's
     private pad row, so their adds are no-ops. This replaces v3's 14
     GPSIMD ap_gathers (~100us each) entirely.

Host does geometry/index/weight prep (top-k selection, projection, bilinear
weights, scatter cell indices) and final slice assembly (split + transpose).
"""

import dataclasses

import numpy as np

# ---------------- constants (hardcoded from the problem spec) ----------------
B = 2
N_PTS = 32
C = 64
H_IMG, W_IMG = 96, 320
HW = H_IMG * W_IMG
X_L, Y_L = 432, 496
VX = VY = np.float32(0.16)
X_OFF = np.float32(0.16 / 2 + 0.0)
Y_OFF = np.float32(0.16 / 2 + (-39.68))
K_TOP = 5
BN_EPS = np.float32(1e-3)

N_CORES = 8
N_REG = 4                      # y-regions per batch
ROWS_REG = Y_L // N_REG        # 124
CELLS = ROWS_REG * X_L         # 53568
PMAX = 3072                    # max pillars per core (padded), 24*128
HALF = PMAX // 2               # 1536
FT_COLS = PMAX * N_PTS         # 98304 conv feature columns
NG = 10                        # img gathers: 5 points x 2 y-rows
PB = PMAX // 128               # 24 slot blocks

NWIN = 4                       # scatter windows (int16 cell-index range)
WROWS = CELLS // NWIN          # 13392 canvas rows per window
WSLOTS = PMAX // NWIN          # 768 slots per window
WBLK = PB // NWIN              # 6 vt blocks per window
ROWS_PAD = WROWS + 1           # +1 private pad row per window
OUT_ROWS = NWIN * ROWS_PAD     # 53572

_compiled = [None]


# =============================== device kernel ===============================
def _build():
    import concourse.tile as tile
    import concourse.bacc as bacc
    from concourse import mybir
    from concourse.masks import make_identity

    f32 = mybir.dt.float32
    bf16 = mybir.dt.bfloat16
    i16 = mybir.dt.int16

    nc = bacc.Bacc("TRN2", target_bir_lowering=False, debug=False,
                   num_devices=N_CORES, num_swdge_queues=4)

    featsT = nc.dram_tensor("featsT", [10, FT_COLS], bf16, kind="ExternalInput")
    lhsT_d = nc.dram_tensor("lhsT", [10, 256], bf16, kind="ExternalInput")
    # image rows padded to 128 bf16 elems (256B) so the gather row stride
    # satisfies the 256B-multiple constraint; data in elems 0:64 of each row.
    img_d = nc.dram_tensor("img", [HW + 2, 2 * C], bf16, kind="ExternalInput")
    gidx_d = nc.dram_tensor("gidx", [128, NG * (PMAX // 16)], i16,
                            kind="ExternalInput")
    gw2_d = nc.dram_tensor("gw2", [128, NG * PB * 2], bf16,
                           kind="ExternalInput")
    widx_d = nc.dram_tensor("widx", [128, NWIN * (WSLOTS // 16)], i16,
                            kind="ExternalInput")
    out_li = nc.dram_tensor("out_li", [OUT_ROWS, 128], bf16,
                            kind="ExternalOutput")

    # paired-row gather view: row stride 128 elems, 256-elem (2-row) reads
    img_pair_ap = dataclasses.replace(img_d[:, :],
                                      ap=[[2 * C, HW], [1, 4 * C]])

    with tile.TileContext(nc) as tc:
        with (
            tc.tile_pool(name="const", bufs=1) as cpool,
            tc.tile_pool(name="zt", bufs=1) as zpool,
            tc.tile_pool(name="ft", bufs=3) as ftpool,
            tc.tile_pool(name="gimg", bufs=4) as gpool,
            tc.tile_pool(name="tmp", bufs=2) as tpool,
            tc.tile_pool(name="acc", bufs=1) as apool,
            tc.tile_pool(name="psA", bufs=3, space="PSUM") as psA,
            tc.tile_pool(name="psT", bufs=2, space="PSUM") as psT,
        ):
            # ---- constants / small loads (before zero-fill so the first
            # gather isn't stuck behind 13MB of zeros on the DMA queues) ----
            ident = cpool.tile([128, 128], bf16)
            make_identity(nc, ident[:])
            lhsT = cpool.tile([10, 256], bf16)
            nc.sync.dma_start(lhsT[:], lhsT_d[:])
            gidx = cpool.tile([128, NG * (PMAX // 16)], i16)
            nc.sync.dma_start(gidx[:], gidx_d[:])
            gw2 = cpool.tile([128, NG * PB * 2], bf16)
            nc.sync.dma_start(gw2[:], gw2_d[:])
            widx = cpool.tile([128, NWIN * (WSLOTS // 16)], i16)
            nc.sync.dma_start(widx[:], widx_d[:])

            # ---- zero-fill the output canvas (overlaps all compute) ----
            zt = zpool.tile([128, ROWS_PAD], bf16)
            nc.vector.memset(zt[:], 0.0)
            for i in range(NWIN):
                dst = dataclasses.replace(
                    out_li[:, :], ap=[[ROWS_PAD, 128], [1, ROWS_PAD]],
                    offset=i * 128 * ROWS_PAD)
                nc.scalar.dma_start(dst, zt[:])

            # slot-major pillar table: [slot%128, slot//128, 128ch]
            vt = apool.tile([128, PB, 128], bf16)
            pooled = apool.tile([128, HALF], bf16)

            # ---- phase C: conv+BN matmul (bf16), maxpool ----
            for u in range(FT_COLS // 4096):
                ft = ftpool.tile([10, 4096], bf16)
                nc.sync.dma_start(ft[:], featsT[:, u * 4096:(u + 1) * 4096])
                for tt in range(2):
                    t = 2 * u + tt
                    h = psA.tile([128, 1024], f32, space="PSUM")
                    for q in range(2):
                        base = tt * 2048 + q * 1024
                        nc.tensor.matmul(h[:, q * 512:(q + 1) * 512],
                                         lhsT[:, 0:128],
                                         ft[:, base:base + 512],
                                         start=True, stop=False)
                        nc.tensor.matmul(h[:, q * 512:(q + 1) * 512],
                                         lhsT[:, 128:256],
                                         ft[:, base + 512:base + 1024],
                                         start=False, stop=True)
                    nc.vector.reduce_max(
                        pooled[:, 32 * t:32 * (t + 1)],
                        h[:].rearrange("p (i n) -> p i n", n=N_PTS),
                        axis=mybir.AxisListType.X)

            # ---- transpose pooled into slot-major vt (relu via Act copy) ---
            # pooled col j = slot j (rows 0:64) / slot 1536+j (rows 64:128)
            for w in range(12):
                pv = psT.tile([128, 128], bf16, space="PSUM")
                nc.tensor.transpose(pv[:], pooled[:, 128 * w:128 * (w + 1)],
                                    ident[:])
                nc.scalar.activation(vt[:, w, 0:64], pv[:, 0:64],
                                     func=mybir.ActivationFunctionType.Relu)
                nc.scalar.activation(vt[:, 12 + w, 0:64], pv[:, 64:128],
                                     func=mybir.ActivationFunctionType.Relu)

            # ---- phase E: bilinear image features (slot-major) ----
            acc = apool.tile([128, PB, C], f32)
            qmap = [0, 1, 2, 3, 0, 1, 2, 3, 0, 1]
            for g in range(NG):
                gt = gpool.tile([128, PB, 4 * C], bf16, tag="g")
                nc.gpsimd.dma_gather(
                    gt[:], img_pair_ap,
                    gidx[:, g * (PMAX // 16):(g + 1) * (PMAX // 16)],
                    PMAX, PMAX, 4 * C, elem_step=2 * C,
                    single_packet=False, queue_num=qmap[g])
                tmp = tpool.tile([128, PB, 2, C], bf16, tag="t")
                wap = (gw2[:, g * 2 * PB:(g + 1) * 2 * PB]
                       .broadcast_to([128, 2 * PB, C]))
                # x0 cell at elems 0:64 of each 128-elem half-row, x1 at
                # 128:192; (u, half) fuses to one stride-128 dim of size 48.
                nc.vector.tensor_tensor(
                    tmp[:].rearrange("p u h c -> p (u h) c"),
                    gt[:].rearrange("p u (h c) -> p (u h) c", c=2 * C)
                         [:, :, 0:C],
                    wap, op=mybir.AluOpType.mult)
                if g == 0:
                    nc.vector.tensor_tensor(
                        acc[:], tmp[:, :, 0, :], tmp[:, :, 1, :],
                        op=mybir.AluOpType.add)
                else:
                    nc.vector.tensor_tensor(
                        acc[:], acc[:], tmp[:, :, 0, :],
                        op=mybir.AluOpType.add)
                    dst = (vt[:, :, C:2 * C] if g == NG - 1 else acc[:])
                    nc.vector.tensor_tensor(
                        dst, acc[:], tmp[:, :, 1, :],
                        op=mybir.AluOpType.add)

            # ---- phase F: sparse scatter-add of pillar rows onto the zeroed
            # canvas. One call per 13392-row window keeps indices in int16.
            # SWDGE sem lanes are assigned round-robin (mod 8) over Pool DMA
            # instructions and each lane is locked to one queue: gathers are
            # lanes 0..7,0,1 (queues qmap), scatters lanes 2,3,4,5 must reuse
            # those lanes' queues -> [2, 3, 0, 1].
            for c in range(NWIN):
                dstw = dataclasses.replace(
                    out_li[:, :], ap=[[128, ROWS_PAD], [1, 128]],
                    offset=c * ROWS_PAD * 128)
                nc.gpsimd.dma_scatter_add(
                    dstw, vt[:, WBLK * c:WBLK * (c + 1), :],
                    widx[:, (WSLOTS // 16) * c:(WSLOTS // 16) * (c + 1)],
                    WSLOTS, WSLOTS, 128, elem_step=128,
                    single_packet=False, queue_num=[2, 3, 0, 1][c])

    nc.compile()
    return nc


def _get_nc():
    if _compiled[0] is None:
        _compiled[0] = _build()
    return _compiled[0]


# ================================ host prep =================================
def _slot_cols(slots):
    """featsT column base for each slot (conv psum-tile layout)."""
    s = np.asarray(slots)
    h = (s >= HALF).astype(np.int64)
    sl = s - h * HALF
    t = sl // 16
    i = sl % 16
    return 1024 * t + 512 * h + 32 * i


def _wrap16(a):
    """[N] -> [128, N//16] wrapped-16 index layout (replicated x8)."""
    n = a.shape[0]
    w = a.reshape(n // 16, 16).T
    return np.tile(w, (8, 1))


def _host_prep(pillars, coors_batch, npoints_per_pillar, batched_image_map,
               image_shapes, proj_mats, conv_w, bn_gamma, bn_beta, bn_mean,
               bn_var):
    from concourse import mybir
    np_bf16 = mybir.dt.np(mybir.dt.bfloat16)

    pillars = np.asarray(pillars, np.float32)
    coors = np.asarray(coors_batch, np.int64)
    npoints = np.asarray(npoints_per_pillar, np.int64)
    img = np.asarray(batched_image_map, np.float32)
    ish = np.asarray(image_shapes, np.int64)
    projm = np.asarray(proj_mats, np.float32)
    conv_w = np.asarray(conv_w, np.float32)
    bn_gamma = np.asarray(bn_gamma, np.float32)
    bn_beta = np.asarray(bn_beta, np.float32)
    bn_mean = np.asarray(bn_mean, np.float32)
    bn_var = np.asarray(bn_var, np.float32)

    xyz = pillars[:, :, :3]
    npf = npoints.astype(np.float32)
    mean = xyz.sum(axis=1) / npf[:, None]
    offset = xyz - mean[:, None, :]
    mask = np.arange(N_PTS)[None, :] < npoints[:, None]
    cx = coors[:, 1].astype(np.float32) * VX + X_OFF
    cy = coors[:, 2].astype(np.float32) * VY + Y_OFF
    x_off = pillars[:, :, 0] - cx[:, None]
    y_off = pillars[:, :, 1] - cy[:, None]
    feats9 = np.stack([x_off, y_off, pillars[:, :, 2], pillars[:, :, 3],
                       offset[:, :, 0], offset[:, :, 1], offset[:, :, 2],
                       x_off, y_off], axis=-1)
    feats9 = feats9 * mask[:, :, None].astype(np.float32)

    # top-K closest valid points (stable ties like lax.top_k)
    dists = np.where(mask, (offset * offset).sum(-1), np.float32(1e6))
    tidx = np.argsort(dists, axis=1, kind="stable")[:, :K_TOP]
    tmask = np.take_along_axis(mask, tidx, axis=1)
    txyz = np.take_along_axis(xyz, tidx[..., None], axis=1)

    b_idx = coors[:, 0]
    Mb = projm[b_idx]                                    # (P,3,4)
    hom = np.concatenate([txyz, np.ones_like(txyz[..., :1])], axis=-1)
    proj = np.einsum("pij,pkj->pki", Mb, hom).astype(np.float32)
    u = proj[..., 0] / proj[..., 2]
    v = proj[..., 1] / proj[..., 2]
    shf = ish[b_idx].astype(np.float32)                  # (P,2) H_img,W_img
    u_s = u * (np.float32(W_IMG) / shf[:, 1:2])
    v_s = v * (np.float32(H_IMG) / shf[:, 0:1])
    inside = (tmask & (u_s >= 0) & (u_s <= W_IMG - 1)
              & (v_s >= 0) & (v_s <= H_IMG - 1))
    u_c = np.clip(u_s, np.float32(0.0), np.float32(W_IMG - 1))
    v_c = np.clip(v_s, np.float32(0.0), np.float32(H_IMG - 1))
    x0 = np.floor(u_c).astype(np.int32)
    y0 = np.floor(v_c).astype(np.int32)
    y1 = np.minimum(y0 + 1, H_IMG - 1)
    wx = u_c - x0.astype(np.float32)
    wy = v_c - y0.astype(np.float32)
    w_in = inside.astype(np.float64)
    cnt = np.maximum(w_in.sum(axis=1), 1.0)
    wk = w_in / cnt[:, None]                             # (P,K)
    wx64, wy64 = wx.astype(np.float64), wy.astype(np.float64)
    # per (point, y-row h, x-half): folded bilinear weights
    cw = (np.stack([(1 - wx64) * (1 - wy64), wx64 * (1 - wy64),
                    (1 - wx64) * wy64, wx64 * wy64], axis=-1)
          * wk[..., None]).astype(np.float32)            # (P,K,4): 00,01,10,11
    rows_pair = np.stack([y0 * W_IMG + x0, y1 * W_IMG + x0], -1)  # (P,K,2)

    # folded conv+BN
    scale = bn_gamma / np.sqrt(bn_var + BN_EPS)
    Wp = conv_w * scale[:, None]                         # (64,9)
    bp = bn_beta - bn_mean * scale
    lhsT = np.zeros((10, 256), np.float32)
    lhsT[0:9, 0:64] = Wp.T
    lhsT[9, 0:64] = bp
    lhsT[0:9, 192:256] = Wp.T
    lhsT[9, 192:256] = bp
    lhsT = lhsT.astype(np_bf16)

    xc = coors[:, 1].astype(np.int64)
    yc = coors[:, 2].astype(np.int64)

    in_maps = []
    for core in range(N_CORES):
        b, r = divmod(core, N_REG)
        sel = np.where((b_idx == b) & (yc >= ROWS_REG * r)
                       & (yc < ROWS_REG * (r + 1)))[0]
        lin = (yc[sel] - ROWS_REG * r) * X_L + xc[sel]
        order = np.argsort(lin, kind="stable")
        sel = sel[order]
        lin = lin[order]
        n_p = len(sel)
        assert n_p <= PMAX, f"core {core}: {n_p} pillars > PMAX {PMAX}"

        # window-packed slot assignment: window c gets slots
        # [WSLOTS*c, WSLOTS*(c+1)); pads keep zero V rows and target the
        # window's private pad row WROWS.
        win = lin // WROWS                               # 0..3, sorted
        rel = (lin - win * WROWS).astype(np.int16)
        slot = np.empty(n_p, np.int64)
        widx = np.full(PMAX, WROWS, np.int16)
        for c in range(NWIN):
            m = np.nonzero(win == c)[0]
            assert len(m) <= WSLOTS, \
                f"core {core} window {c}: {len(m)} pillars > {WSLOTS}"
            slot[m] = WSLOTS * c + np.arange(len(m))
            widx[WSLOTS * c:WSLOTS * c + len(m)] = rel[m]

        ftT = np.zeros((10, FT_COLS), np_bf16)
        colb = _slot_cols(slot)
        cols = (colb[:, None] + np.arange(N_PTS)[None, :]).ravel()
        ftT[9, cols] = 1.0
        ftT[0:9].reshape(9, FT_COLS)[:, cols] = (
            feats9[sel].transpose(2, 0, 1).reshape(9, n_p * N_PTS)
            .astype(np_bf16))

        # gather indices (pads fetch row 0, weight 0) + folded weights
        gi = np.zeros((NG, PMAX), np.int16)
        gwv = np.zeros((NG, PB * 2, 128), np.float32)
        for k in range(K_TOP):
            for hf in range(2):
                g = k * 2 + hf
                gi[g, slot] = rows_pair[sel, k, hf].astype(np.int16)
                wlo = np.zeros(PMAX, np.float32)
                whi = np.zeros(PMAX, np.float32)
                wlo[slot] = cw[sel, k, 2 * hf]        # x0 cell
                whi[slot] = cw[sel, k, 2 * hf + 1]    # x1 cell
                gwv[g, 0::2, :] = wlo.reshape(PB, 128)
                gwv[g, 1::2, :] = whi.reshape(PB, 128)
        gidx = _wrap16(gi.reshape(NG * PMAX)).reshape(
            128, NG, PMAX // 16).reshape(128, NG * (PMAX // 16))
        gw2 = (gwv.reshape(NG * PB * 2, 128).T           # (128, NG*PB*2)
               .astype(np_bf16))

        imgt = np.zeros((HW + 2, 2 * C), np_bf16)
        imgt[:HW, :C] = (img[b].transpose(1, 2, 0).reshape(HW, C)
                         .astype(np_bf16))

        in_maps.append({
            "featsT": ftT,
            "lhsT": lhsT,
            "img": imgt,
            "gidx": np.ascontiguousarray(gidx),
            "gw2": np.ascontiguousarray(gw2),
            "widx": np.ascontiguousarray(_wrap16(widx)),
        })
    return in_maps


def _assemble(results):
    canvas_l = np.zeros((B, C, Y_L, X_L), np.float32)
    canvas_i = np.zeros((B, C, Y_L, X_L), np.float32)
    for core in range(N_CORES):
        b, r = divmod(core, N_REG)
        ysl = slice(ROWS_REG * r, ROWS_REG * (r + 1))
        arr = (results[core]["out_li"].reshape(NWIN, ROWS_PAD, 128)[:, :WROWS]
               .reshape(CELLS, 128).astype(np.float32))
        canvas_l[b, :, ysl, :] = arr[:, 0:C].T.reshape(C, ROWS_REG, X_L)
        canvas_i[b, :, ysl, :] = arr[:, C:2 * C].T.reshape(C, ROWS_REG, X_L)
    return canvas_l, canvas_i


def kernel(**inputs):
    from concourse.bass_utils import run_bass_kernel_spmd

    nc = _get_nc()
    in_maps = _host_prep(**inputs)
    res = run_bass_kernel_spmd(nc, in_maps, list(range(N_CORES)))
    return _assemble(res.results)


# revision 19
# speedup vs baseline: 7.5007x; 1.1109x over previous
"""PillarEncoder Trainium2 kernel (v4).

Strategy (8 NeuronCores, pure SPMD, no collectives):
  core = (batch b in {0,1}) x (canvas y-region r in {0..3}, 124 rows each).
  Host bins each batch's pillars by output-canvas y-row into the owning core,
  so every core computes features for exactly the pillars that land in its
  output slice; no cross-core exchange is needed.

Per core, on device (all heavy data paths redesigned vs v3):
  1. conv(1x1)+BN folded into one K=10 matmul in bf16 (fp32 matmul costs
     4 cyc/row on TRN2 PE; bf16 costs 1). relu folded into the later
     Activation-engine copies; maxpool = DVE reduce_max on PSUM.
  2. image bilinear sampling: 10 dma_gathers (5 top-k points x 2 y-rows,
     each fetching the x0/x1 cell pair in one 512B descriptor). Bilinear
     weights are applied with broadcast-AP tensor_tensor ops (w varies
     along the free dim), ~3 DVE ops per gather instead of 48
     scalar_tensor_tensor ops.
  3. output: a slot-major pillar table V [slot(part) x 128ch] in bf16
     (cols 0:64 pooled, 64:128 image feat). The BEV canvases live in DRAM
     as one [4 windows x (13392+1 pad) rows, 128ch] bf16 tensor: big DMAs
     zero-fill it at kernel start (overlapped under compute), then 4 SWDGE
     dma_scatter_add calls (one per 13392-row window so cell indices fit
     int16) add the ~2.5k pillar rows (256B descriptors) onto the zeroed
     canvas. Pad slots carry exactly-zero V rows and target each window's
     private pad row, so their adds are no-ops. This replaces v3's 14
     GPSIMD ap_gathers (~100us each) entirely.

Host does geometry/index/weight prep (top-k selection, projection, bilinear
weights, scatter cell indices) and final slice assembly (split + transpose).
"""

import dataclasses

import numpy as np

# ---------------- constants (hardcoded from the problem spec) ----------------
B = 2
N_PTS = 32
C = 64
H_IMG, W_IMG = 96, 320
HW = H_IMG * W_IMG
X_L, Y_L = 432, 496
VX = VY = np.float32(0.16)
X_OFF = np.float32(0.16 / 2 + 0.0)
Y_OFF = np.float32(0.16 / 2 + (-39.68))
K_TOP = 5
BN_EPS = np.float32(1e-3)

N_CORES = 8
N_REG = 4                      # y-regions per batch
ROWS_REG = Y_L // N_REG        # 124
CELLS = ROWS_REG * X_L         # 53568
PMAX = 3072                    # max pillars per core (padded), 24*128
HALF = PMAX // 2               # 1536
FT_COLS = PMAX * N_PTS         # 98304 conv feature columns
NG = 10                        # img gathers: 5 points x 2 y-rows
PB = PMAX // 128               # 24 slot blocks

NWIN = 4                       # scatter windows (int16 cell-index range)
WROWS = CELLS // NWIN          # 13392 canvas rows per window
WSLOTS = PMAX // NWIN          # 768 slots per window
WBLK = PB // NWIN              # 6 vt blocks per window
ROWS_PAD = WROWS + 1           # +1 private pad row per window
OUT_ROWS = NWIN * ROWS_PAD     # 53572

_compiled = [None]


# =============================== device kernel ===============================
def _build():
    import concourse.tile as tile
    import concourse.bacc as bacc
    from concourse import mybir
    from concourse.masks import make_identity

    f32 = mybir.dt.float32
    bf16 = mybir.dt.bfloat16
    i16 = mybir.dt.int16

    nc = bacc.Bacc("TRN2", target_bir_lowering=False, debug=False,
                   num_devices=N_CORES, num_swdge_queues=4,
                   dynamic_dma_scratch_size=32768)

    featsT = nc.dram_tensor("featsT", [10, FT_COLS], bf16, kind="ExternalInput")
    lhsT_d = nc.dram_tensor("lhsT", [10, 256], bf16, kind="ExternalInput")
    # image rows padded to 128 bf16 elems (256B) so the gather row stride
    # satisfies the 256B-multiple constraint; data in elems 0:64 of each row.
    img_d = nc.dram_tensor("img", [HW + 2, 2 * C], bf16, kind="ExternalInput")
    gidx_d = nc.dram_tensor("gidx", [128, NG * (PMAX // 16)], i16,
                            kind="ExternalInput")
    gw2_d = nc.dram_tensor("gw2", [128, NG * PB * 2], bf16,
                           kind="ExternalInput")
    widx_d = nc.dram_tensor("widx", [128, NWIN * (WSLOTS // 16)], i16,
                            kind="ExternalInput")
    out_li = nc.dram_tensor("out_li", [OUT_ROWS, 128], bf16,
                            kind="ExternalOutput")

    # paired-row gather view: row stride 128 elems, 256-elem (2-row) reads
    img_pair_ap = dataclasses.replace(img_d[:, :],
                                      ap=[[2 * C, HW], [1, 4 * C]])

    with tile.TileContext(nc) as tc:
        with (
            tc.tile_pool(name="const", bufs=1) as cpool,
            tc.tile_pool(name="zt", bufs=1) as zpool,
            tc.tile_pool(name="ft", bufs=3) as ftpool,
            tc.tile_pool(name="gimg", bufs=4) as gpool,
            tc.tile_pool(name="tmp", bufs=2) as tpool,
            tc.tile_pool(name="acc", bufs=1) as apool,
            tc.tile_pool(name="psA", bufs=3, space="PSUM") as psA,
            tc.tile_pool(name="psT", bufs=2, space="PSUM") as psT,
        ):
            # ---- constants / small loads (before zero-fill so the first
            # gather isn't stuck behind 13MB of zeros on the DMA queues) ----
            ident = cpool.tile([128, 128], bf16)
            make_identity(nc, ident[:])
            lhsT = cpool.tile([10, 256], bf16)
            nc.sync.dma_start(lhsT[:], lhsT_d[:])
            gidx = cpool.tile([128, NG * (PMAX // 16)], i16)
            nc.sync.dma_start(gidx[:], gidx_d[:])
            gw2 = cpool.tile([128, NG * PB * 2], bf16)
            nc.sync.dma_start(gw2[:], gw2_d[:])
            widx = cpool.tile([128, NWIN * (WSLOTS // 16)], i16)
            nc.sync.dma_start(widx[:], widx_d[:])

            # ---- zero-fill the output canvas (overlaps all compute) ----
            zt = zpool.tile([128, ROWS_PAD], bf16)
            nc.vector.memset(zt[:], 0.0)
            for i in range(NWIN):
                dst = dataclasses.replace(
                    out_li[:, :], ap=[[ROWS_PAD, 128], [1, ROWS_PAD]],
                    offset=i * 128 * ROWS_PAD)
                nc.scalar.dma_start(dst, zt[:])

            # slot-major pillar table: [slot%128, slot//128, 128ch]
            vt = apool.tile([128, PB, 128], bf16)
            pooled = apool.tile([128, HALF], bf16)

            # ---- phase C: conv+BN matmul (bf16), maxpool ----
            for u in range(FT_COLS // 4096):
                ft = ftpool.tile([10, 4096], bf16)
                nc.sync.dma_start(ft[:], featsT[:, u * 4096:(u + 1) * 4096])
                for tt in range(2):
                    t = 2 * u + tt
                    h = psA.tile([128, 1024], f32, space="PSUM")
                    for q in range(2):
                        base = tt * 2048 + q * 1024
                        nc.tensor.matmul(h[:, q * 512:(q + 1) * 512],
                                         lhsT[:, 0:128],
                                         ft[:, base:base + 512],
                                         start=True, stop=False)
                        nc.tensor.matmul(h[:, q * 512:(q + 1) * 512],
                                         lhsT[:, 128:256],
                                         ft[:, base + 512:base + 1024],
                                         start=False, stop=True)
                    nc.vector.reduce_max(
                        pooled[:, 32 * t:32 * (t + 1)],
                        h[:].rearrange("p (i n) -> p i n", n=N_PTS),
                        axis=mybir.AxisListType.X)

            # ---- transpose pooled into slot-major vt (relu via Act copy) ---
            # pooled col j = slot j (rows 0:64) / slot 1536+j (rows 64:128)
            for w in range(12):
                pv = psT.tile([128, 128], bf16, space="PSUM")
                nc.tensor.transpose(pv[:], pooled[:, 128 * w:128 * (w + 1)],
                                    ident[:])
                nc.scalar.activation(vt[:, w, 0:64], pv[:, 0:64],
                                     func=mybir.ActivationFunctionType.Relu)
                nc.scalar.activation(vt[:, 12 + w, 0:64], pv[:, 64:128],
                                     func=mybir.ActivationFunctionType.Relu)

            # ---- phase E: bilinear image features (slot-major) ----
            # Each gather is split into two 1536-desc halves on different
            # SWDGE queues: a 3072-desc prep overflows the descriptor ring
            # and serializes desc-gen behind its own DMA drain. Queue =
            # Pool-DMA position % 4 keeps every sem lane (mod 8) on one
            # queue permanently.
            acc = apool.tile([128, PB, C], f32)
            HB = PMAX // 2 // 16           # idx cols per half (96)
            for g in range(NG):
                gt = gpool.tile([128, PB, 4 * C], bf16, tag="g")
                for hf in range(2):
                    nc.gpsimd.dma_gather(
                        gt[:, hf * (PB // 2):(hf + 1) * (PB // 2), :],
                        img_pair_ap,
                        gidx[:, g * 2 * HB + hf * HB:
                             g * 2 * HB + (hf + 1) * HB],
                        PMAX // 2, PMAX // 2, 4 * C, elem_step=2 * C,
                        single_packet=False, queue_num=(2 * g + hf) % 4)
                tmp = tpool.tile([128, PB, 2, C], bf16, tag="t")
                wap = (gw2[:, g * 2 * PB:(g + 1) * 2 * PB]
                       .broadcast_to([128, 2 * PB, C]))
                # x0 cell at elems 0:64 of each 128-elem half-row, x1 at
                # 128:192; (u, half) fuses to one stride-128 dim of size 48.
                nc.vector.tensor_tensor(
                    tmp[:].rearrange("p u h c -> p (u h) c"),
                    gt[:].rearrange("p u (h c) -> p (u h) c", c=2 * C)
                         [:, :, 0:C],
                    wap, op=mybir.AluOpType.mult)
                if g == 0:
                    nc.vector.tensor_tensor(
                        acc[:], tmp[:, :, 0, :], tmp[:, :, 1, :],
                        op=mybir.AluOpType.add)
                else:
                    nc.vector.tensor_tensor(
                        acc[:], acc[:], tmp[:, :, 0, :],
                        op=mybir.AluOpType.add)
                    dst = (vt[:, :, C:2 * C] if g == NG - 1 else acc[:])
                    nc.vector.tensor_tensor(
                        dst, acc[:], tmp[:, :, 1, :],
                        op=mybir.AluOpType.add)

            # ---- phase F: sparse scatter-add of pillar rows onto the zeroed
            # canvas. One call per 13392-row window keeps indices in int16.
            # Scatters are Pool-DMA #20..23 (lanes 4..7) -> queues 0..3.
            for c in range(NWIN):
                dstw = dataclasses.replace(
                    out_li[:, :], ap=[[128, ROWS_PAD], [1, 128]],
                    offset=c * ROWS_PAD * 128)
                nc.gpsimd.dma_scatter_add(
                    dstw, vt[:, WBLK * c:WBLK * (c + 1), :],
                    widx[:, (WSLOTS // 16) * c:(WSLOTS // 16) * (c + 1)],
                    WSLOTS, WSLOTS, 128, elem_step=128,
                    single_packet=False, queue_num=c)

    nc.compile()
    return nc


def _get_nc():
    if _compiled[0] is None:
        _compiled[0] = _build()
    return _compiled[0]


# ================================ host prep =================================
def _slot_cols(slots):
    """featsT column base for each slot (conv psum-tile layout)."""
    s = np.asarray(slots)
    h = (s >= HALF).astype(np.int64)
    sl = s - h * HALF
    t = sl // 16
    i = sl % 16
    return 1024 * t + 512 * h + 32 * i


def _wrap16(a):
    """[N] -> [128, N//16] wrapped-16 index layout (replicated x8)."""
    n = a.shape[0]
    w = a.reshape(n // 16, 16).T
    return np.tile(w, (8, 1))


def _host_prep(pillars, coors_batch, npoints_per_pillar, batched_image_map,
               image_shapes, proj_mats, conv_w, bn_gamma, bn_beta, bn_mean,
               bn_var):
    from concourse import mybir
    np_bf16 = mybir.dt.np(mybir.dt.bfloat16)

    pillars = np.asarray(pillars, np.float32)
    coors = np.asarray(coors_batch, np.int64)
    npoints = np.asarray(npoints_per_pillar, np.int64)
    img = np.asarray(batched_image_map, np.float32)
    ish = np.asarray(image_shapes, np.int64)
    projm = np.asarray(proj_mats, np.float32)
    conv_w = np.asarray(conv_w, np.float32)
    bn_gamma = np.asarray(bn_gamma, np.float32)
    bn_beta = np.asarray(bn_beta, np.float32)
    bn_mean = np.asarray(bn_mean, np.float32)
    bn_var = np.asarray(bn_var, np.float32)

    xyz = pillars[:, :, :3]
    npf = npoints.astype(np.float32)
    mean = xyz.sum(axis=1) / npf[:, None]
    offset = xyz - mean[:, None, :]
    mask = np.arange(N_PTS)[None, :] < npoints[:, None]
    cx = coors[:, 1].astype(np.float32) * VX + X_OFF
    cy = coors[:, 2].astype(np.float32) * VY + Y_OFF
    x_off = pillars[:, :, 0] - cx[:, None]
    y_off = pillars[:, :, 1] - cy[:, None]
    feats9 = np.stack([x_off, y_off, pillars[:, :, 2], pillars[:, :, 3],
                       offset[:, :, 0], offset[:, :, 1], offset[:, :, 2],
                       x_off, y_off], axis=-1)
    feats9 = feats9 * mask[:, :, None].astype(np.float32)

    # top-K closest valid points (stable ties like lax.top_k)
    dists = np.where(mask, (offset * offset).sum(-1), np.float32(1e6))
    tidx = np.argsort(dists, axis=1, kind="stable")[:, :K_TOP]
    tmask = np.take_along_axis(mask, tidx, axis=1)
    txyz = np.take_along_axis(xyz, tidx[..., None], axis=1)

    b_idx = coors[:, 0]
    Mb = projm[b_idx]                                    # (P,3,4)
    hom = np.concatenate([txyz, np.ones_like(txyz[..., :1])], axis=-1)
    proj = np.einsum("pij,pkj->pki", Mb, hom).astype(np.float32)
    u = proj[..., 0] / proj[..., 2]
    v = proj[..., 1] / proj[..., 2]
    shf = ish[b_idx].astype(np.float32)                  # (P,2) H_img,W_img
    u_s = u * (np.float32(W_IMG) / shf[:, 1:2])
    v_s = v * (np.float32(H_IMG) / shf[:, 0:1])
    inside = (tmask & (u_s >= 0) & (u_s <= W_IMG - 1)
              & (v_s >= 0) & (v_s <= H_IMG - 1))
    u_c = np.clip(u_s, np.float32(0.0), np.float32(W_IMG - 1))
    v_c = np.clip(v_s, np.float32(0.0), np.float32(H_IMG - 1))
    x0 = np.floor(u_c).astype(np.int32)
    y0 = np.floor(v_c).astype(np.int32)
    y1 = np.minimum(y0 + 1, H_IMG - 1)
    wx = u_c - x0.astype(np.float32)
    wy = v_c - y0.astype(np.float32)
    w_in = inside.astype(np.float64)
    cnt = np.maximum(w_in.sum(axis=1), 1.0)
    wk = w_in / cnt[:, None]                             # (P,K)
    wx64, wy64 = wx.astype(np.float64), wy.astype(np.float64)
    # per (point, y-row h, x-half): folded bilinear weights
    cw = (np.stack([(1 - wx64) * (1 - wy64), wx64 * (1 - wy64),
                    (1 - wx64) * wy64, wx64 * wy64], axis=-1)
          * wk[..., None]).astype(np.float32)            # (P,K,4): 00,01,10,11
    rows_pair = np.stack([y0 * W_IMG + x0, y1 * W_IMG + x0], -1)  # (P,K,2)

    # folded conv+BN
    scale = bn_gamma / np.sqrt(bn_var + BN_EPS)
    Wp = conv_w * scale[:, None]                         # (64,9)
    bp = bn_beta - bn_mean * scale
    lhsT = np.zeros((10, 256), np.float32)
    lhsT[0:9, 0:64] = Wp.T
    lhsT[9, 0:64] = bp
    lhsT[0:9, 192:256] = Wp.T
    lhsT[9, 192:256] = bp
    lhsT = lhsT.astype(np_bf16)

    xc = coors[:, 1].astype(np.int64)
    yc = coors[:, 2].astype(np.int64)

    in_maps = []
    for core in range(N_CORES):
        b, r = divmod(core, N_REG)
        sel = np.where((b_idx == b) & (yc >= ROWS_REG * r)
                       & (yc < ROWS_REG * (r + 1)))[0]
        lin = (yc[sel] - ROWS_REG * r) * X_L + xc[sel]
        order = np.argsort(lin, kind="stable")
        sel = sel[order]
        lin = lin[order]
        n_p = len(sel)
        assert n_p <= PMAX, f"core {core}: {n_p} pillars > PMAX {PMAX}"

        # window-packed slot assignment: window c gets slots
        # [WSLOTS*c, WSLOTS*(c+1)); pads keep zero V rows and target the
        # window's private pad row WROWS.
        win = lin // WROWS                               # 0..3, sorted
        rel = (lin - win * WROWS).astype(np.int16)
        slot = np.empty(n_p, np.int64)
        widx = np.full(PMAX, WROWS, np.int16)
        for c in range(NWIN):
            m = np.nonzero(win == c)[0]
            assert len(m) <= WSLOTS, \
                f"core {core} window {c}: {len(m)} pillars > {WSLOTS}"
            slot[m] = WSLOTS * c + np.arange(len(m))
            widx[WSLOTS * c:WSLOTS * c + len(m)] = rel[m]

        ftT = np.zeros((10, FT_COLS), np_bf16)
        colb = _slot_cols(slot)
        cols = (colb[:, None] + np.arange(N_PTS)[None, :]).ravel()
        ftT[9, cols] = 1.0
        ftT[0:9].reshape(9, FT_COLS)[:, cols] = (
            feats9[sel].transpose(2, 0, 1).reshape(9, n_p * N_PTS)
            .astype(np_bf16))

        # gather indices (pads fetch row 0, weight 0) + folded weights
        gi = np.zeros((NG, PMAX), np.int16)
        gwv = np.zeros((NG, PB * 2, 128), np.float32)
        for k in range(K_TOP):
            for hf in range(2):
                g = k * 2 + hf
                gi[g, slot] = rows_pair[sel, k, hf].astype(np.int16)
                wlo = np.zeros(PMAX, np.float32)
                whi = np.zeros(PMAX, np.float32)
                wlo[slot] = cw[sel, k, 2 * hf]        # x0 cell
                whi[slot] = cw[sel, k, 2 * hf + 1]    # x1 cell
                gwv[g, 0::2, :] = wlo.reshape(PB, 128)
                gwv[g, 1::2, :] = whi.reshape(PB, 128)
        gidx = _wrap16(gi.reshape(NG * PMAX)).reshape(
            128, NG, PMAX // 16).reshape(128, NG * (PMAX // 16))
        gw2 = (gwv.reshape(NG * PB * 2, 128).T           # (128, NG*PB*2)
               .astype(np_bf16))

        imgt = np.zeros((HW + 2, 2 * C), np_bf16)
        imgt[:HW, :C] = (img[b].transpose(1, 2, 0).reshape(HW, C)
                         .astype(np_bf16))

        in_maps.append({
            "featsT": ftT,
            "lhsT": lhsT,
            "img": imgt,
            "gidx": np.ascontiguousarray(gidx),
            "gw2": np.ascontiguousarray(gw2),
            "widx": np.ascontiguousarray(_wrap16(widx)),
        })
    return in_maps


def _assemble(results):
    canvas_l = np.zeros((B, C, Y_L, X_L), np.float32)
    canvas_i = np.zeros((B, C, Y_L, X_L), np.float32)
    for core in range(N_CORES):
        b, r = divmod(core, N_REG)
        ysl = slice(ROWS_REG * r, ROWS_REG * (r + 1))
        arr = (results[core]["out_li"].reshape(NWIN, ROWS_PAD, 128)[:, :WROWS]
               .reshape(CELLS, 128).astype(np.float32))
        canvas_l[b, :, ysl, :] = arr[:, 0:C].T.reshape(C, ROWS_REG, X_L)
        canvas_i[b, :, ysl, :] = arr[:, C:2 * C].T.reshape(C, ROWS_REG, X_L)
    return canvas_l, canvas_i


def kernel(**inputs):
    from concourse.bass_utils import run_bass_kernel_spmd

    nc = _get_nc()
    in_maps = _host_prep(**inputs)
    res = run_bass_kernel_spmd(nc, in_maps, list(range(N_CORES)))
    return _assemble(res.results)


# revision 24
# speedup vs baseline: 8.2079x; 1.0943x over previous
"""PillarEncoder Trainium2 kernel (v4).

Strategy (8 NeuronCores, pure SPMD, no collectives):
  core = (batch b in {0,1}) x (canvas y-region r in {0..3}, 124 rows each).
  Host bins each batch's pillars by output-canvas y-row into the owning core,
  so every core computes features for exactly the pillars that land in its
  output slice; no cross-core exchange is needed.

Per core, on device (all heavy data paths redesigned vs v3):
  1. conv(1x1)+BN folded into one K=10 matmul in bf16 (fp32 matmul costs
     4 cyc/row on TRN2 PE; bf16 costs 1). relu folded into the later
     Activation-engine copies; maxpool = DVE reduce_max on PSUM.
  2. image bilinear sampling: 10 dma_gathers (5 top-k points x 2 y-rows,
     each fetching the x0/x1 cell pair in one 512B descriptor). Bilinear
     weights are applied with broadcast-AP tensor_tensor ops (w varies
     along the free dim), ~3 DVE ops per gather instead of 48
     scalar_tensor_tensor ops.
  3. output: a slot-major pillar table V [slot(part) x 128ch] in bf16
     (cols 0:64 pooled, 64:128 image feat). The BEV canvases live in DRAM
     as one [4 windows x (13392+1 pad) rows, 128ch] bf16 tensor: big DMAs
     zero-fill it at kernel start (overlapped under compute), then 4 SWDGE
     dma_scatter_add calls (one per 13392-row window so cell indices fit
     int16) add the ~2.5k pillar rows (256B descriptors) onto the zeroed
     canvas. Pad slots carry exactly-zero V rows and target each window's
     private pad row, so their adds are no-ops. This replaces v3's 14
     GPSIMD ap_gathers (~100us each) entirely.

Host does geometry/index/weight prep (top-k selection, projection, bilinear
weights, scatter cell indices) and final slice assembly (split + transpose).
"""

import dataclasses

import numpy as np

# ---------------- constants (hardcoded from the problem spec) ----------------
B = 2
N_PTS = 32
C = 64
H_IMG, W_IMG = 96, 320
HW = H_IMG * W_IMG
X_L, Y_L = 432, 496
VX = VY = np.float32(0.16)
X_OFF = np.float32(0.16 / 2 + 0.0)
Y_OFF = np.float32(0.16 / 2 + (-39.68))
K_TOP = 5
BN_EPS = np.float32(1e-3)

N_CORES = 8
N_REG = 4                      # y-regions per batch
ROWS_REG = Y_L // N_REG        # 124
CELLS = ROWS_REG * X_L         # 53568
PMAX = 3072                    # max pillars per core (padded), 24*128
HALF = PMAX // 2               # 1536
FT_COLS = PMAX * N_PTS         # 98304 conv feature columns
NG = 10                        # img gathers: 5 points x 2 y-rows
PB = PMAX // 128               # 24 slot blocks

NWIN = 4                       # scatter windows (int16 cell-index range)
WROWS = CELLS // NWIN          # 13392 canvas rows per window
WSLOTS = PMAX // NWIN          # 768 slots per window
WBLK = PB // NWIN              # 6 vt blocks per window
ROWS_PAD = WROWS + 1           # +1 private pad row per window
OUT_ROWS = NWIN * ROWS_PAD     # 53572

_compiled = [None]


# =============================== device kernel ===============================
def _build():
    import concourse.tile as tile
    import concourse.bacc as bacc
    from concourse import mybir
    from concourse.masks import make_identity

    f32 = mybir.dt.float32
    bf16 = mybir.dt.bfloat16
    i16 = mybir.dt.int16

    nc = bacc.Bacc("TRN2", target_bir_lowering=False, debug=False,
                   num_devices=N_CORES, num_swdge_queues=4,
                   dynamic_dma_scratch_size=32768)

    featsT = nc.dram_tensor("featsT", [10, FT_COLS], bf16, kind="ExternalInput")
    lhsT_d = nc.dram_tensor("lhsT", [10, 256], bf16, kind="ExternalInput")
    # image rows padded to 128 bf16 elems (256B) so the gather row stride
    # satisfies the 256B-multiple constraint; data in elems 0:64 of each row.
    img_d = nc.dram_tensor("img", [HW + 2, 2 * C], bf16, kind="ExternalInput")
    gidx_d = nc.dram_tensor("gidx", [128, NG * (PMAX // 16)], i16,
                            kind="ExternalInput")
    gw2_d = nc.dram_tensor("gw2", [128, NG * PB * 2], bf16,
                           kind="ExternalInput")
    widx_d = nc.dram_tensor("widx", [128, NWIN * (WSLOTS // 16)], i16,
                            kind="ExternalInput")
    out_li = nc.dram_tensor("out_li", [OUT_ROWS, 128], bf16,
                            kind="ExternalOutput")

    # paired-row gather view: row stride 128 elems, 256-elem (2-row) reads
    img_pair_ap = dataclasses.replace(img_d[:, :],
                                      ap=[[2 * C, HW], [1, 4 * C]])

    with tile.TileContext(nc) as tc:
        with (
            tc.tile_pool(name="const", bufs=1) as cpool,
            tc.tile_pool(name="zt", bufs=1) as zpool,
            tc.tile_pool(name="ft", bufs=3) as ftpool,
            tc.tile_pool(name="gimg", bufs=5) as gpool,
            tc.tile_pool(name="tmp", bufs=2) as tpool,
            tc.tile_pool(name="acc", bufs=1) as apool,
            tc.tile_pool(name="psA", bufs=2, space="PSUM") as psA,
            tc.tile_pool(name="psT", bufs=2, space="PSUM") as psT,
        ):
            # ---- constants / small loads (before zero-fill so the first
            # gather isn't stuck behind 13MB of zeros on the DMA queues) ----
            ident = cpool.tile([128, 128], bf16)
            make_identity(nc, ident[:])
            lhsT = cpool.tile([10, 256], bf16)
            nc.sync.dma_start(lhsT[:], lhsT_d[:])
            gidx = cpool.tile([128, NG * (PMAX // 16)], i16)
            nc.sync.dma_start(gidx[:], gidx_d[:])
            gw2 = cpool.tile([128, NG * PB * 2], bf16)
            nc.sync.dma_start(gw2[:], gw2_d[:])
            widx = cpool.tile([128, NWIN * (WSLOTS // 16)], i16)
            nc.sync.dma_start(widx[:], widx_d[:])

            # ---- zero-fill the output canvas (overlaps all compute) ----
            # 4 sub-DMAs per window from a quarter-size zero tile.
            ZQ = 3349                      # 3*3349 + 3346 = ROWS_PAD
            zt = zpool.tile([128, ZQ], bf16)
            nc.vector.memset(zt[:], 0.0)
            for i in range(NWIN):
                for s in range(4):
                    n = ZQ if s < 3 else ROWS_PAD - 3 * ZQ
                    dst = dataclasses.replace(
                        out_li[:, :], ap=[[n, 128], [1, n]],
                        offset=i * 128 * ROWS_PAD + s * 128 * ZQ)
                    nc.scalar.dma_start(dst, zt[:, 0:n])

            # slot-major pillar table: [slot%128, slot//128, 128ch]
            vt = apool.tile([128, PB, 128], bf16)
            pooled = apool.tile([128, HALF], bf16)

            # ---- phase C: conv+BN matmul (bf16), maxpool ----
            # 1536-col PSUM units (3 banks, bufs=2) -> 32 reduce_max ops
            # instead of 48; psum col == slot col so pooled col j = slot j.
            for u in range(FT_COLS // 6144):
                ft = ftpool.tile([10, 6144], bf16)
                nc.sync.dma_start(ft[:], featsT[:, u * 6144:(u + 1) * 6144])
                for tt in range(2):
                    t = 2 * u + tt
                    h = psA.tile([128, 1536], f32, space="PSUM")
                    for q in range(3):
                        base = tt * 3072 + q * 1024
                        nc.tensor.matmul(h[:, q * 512:(q + 1) * 512],
                                         lhsT[:, 0:128],
                                         ft[:, base:base + 512],
                                         start=True, stop=False)
                        nc.tensor.matmul(h[:, q * 512:(q + 1) * 512],
                                         lhsT[:, 128:256],
                                         ft[:, base + 512:base + 1024],
                                         start=False, stop=True)
                    nc.vector.reduce_max(
                        pooled[:, 48 * t:48 * (t + 1)],
                        h[:].rearrange("p (i n) -> p i n", n=N_PTS),
                        axis=mybir.AxisListType.X)

            # ---- transpose pooled into slot-major vt (relu via Act copy) ---
            # pooled col j = slot j (rows 0:64) / slot 1536+j (rows 64:128)
            for w in range(12):
                pv = psT.tile([128, 128], bf16, space="PSUM")
                nc.tensor.transpose(pv[:], pooled[:, 128 * w:128 * (w + 1)],
                                    ident[:])
                nc.scalar.activation(vt[:, w, 0:64], pv[:, 0:64],
                                     func=mybir.ActivationFunctionType.Relu)
                nc.scalar.activation(vt[:, 12 + w, 0:64], pv[:, 64:128],
                                     func=mybir.ActivationFunctionType.Relu)

            # ---- phase E: bilinear image features (slot-major) ----
            # Each gather is split into two 1536-desc halves on different
            # SWDGE queues: a 3072-desc prep overflows the descriptor ring
            # and serializes desc-gen behind its own DMA drain. Queue =
            # Pool-DMA position % 4 keeps every sem lane (mod 8) on one
            # queue permanently.
            acc = apool.tile([128, PB, C], f32)
            HB = PMAX // 2 // 16           # idx cols per half (96)
            for g in range(NG):
                gt = gpool.tile([128, PB, 4 * C], bf16, tag="g")
                for hf in range(2):
                    nc.gpsimd.dma_gather(
                        gt[:, hf * (PB // 2):(hf + 1) * (PB // 2), :],
                        img_pair_ap,
                        gidx[:, g * 2 * HB + hf * HB:
                             g * 2 * HB + (hf + 1) * HB],
                        PMAX // 2, PMAX // 2, 4 * C, elem_step=2 * C,
                        single_packet=False, queue_num=(2 * g + hf) % 4)
                tmp = tpool.tile([128, PB, 2, C], bf16, tag="t")
                wap = (gw2[:, g * 2 * PB:(g + 1) * 2 * PB]
                       .broadcast_to([128, 2 * PB, C]))
                # x0 cell at elems 0:64 of each 128-elem half-row, x1 at
                # 128:192; (u, half) fuses to one stride-128 dim of size 48.
                nc.vector.tensor_tensor(
                    tmp[:].rearrange("p u h c -> p (u h) c"),
                    gt[:].rearrange("p u (h c) -> p (u h) c", c=2 * C)
                         [:, :, 0:C],
                    wap, op=mybir.AluOpType.mult)
                if g == 0:
                    nc.vector.tensor_tensor(
                        acc[:], tmp[:, :, 0, :], tmp[:, :, 1, :],
                        op=mybir.AluOpType.add)
                elif g < NG - 1:
                    nc.vector.tensor_tensor(
                        acc[:], acc[:], tmp[:, :, 0, :],
                        op=mybir.AluOpType.add)
                    nc.vector.tensor_tensor(
                        acc[:], acc[:], tmp[:, :, 1, :],
                        op=mybir.AluOpType.add)
                else:
                    nc.vector.tensor_tensor(
                        acc[:], acc[:], tmp[:, :, 0, :],
                        op=mybir.AluOpType.add)
                    # final add split per scatter window so each scatter
                    # fires as soon as its vt slice is complete
                    for c in range(NWIN):
                        ws = slice(WBLK * c, WBLK * (c + 1))
                        nc.vector.tensor_tensor(
                            vt[:, ws, C:2 * C], acc[:, ws, :],
                            tmp[:, ws, 1, :], op=mybir.AluOpType.add)

            # ---- phase F: sparse scatter-add of pillar rows onto the zeroed
            # canvas. One call per 13392-row window keeps indices in int16.
            # Scatters are Pool-DMA #20..23 (lanes 4..7) -> queues 0..3.
            for c in range(NWIN):
                dstw = dataclasses.replace(
                    out_li[:, :], ap=[[128, ROWS_PAD], [1, 128]],
                    offset=c * ROWS_PAD * 128)
                nc.gpsimd.dma_scatter_add(
                    dstw, vt[:, WBLK * c:WBLK * (c + 1), :],
                    widx[:, (WSLOTS // 16) * c:(WSLOTS // 16) * (c + 1)],
                    WSLOTS, WSLOTS, 128, elem_step=128,
                    single_packet=False, queue_num=c)

    nc.compile()
    return nc


def _get_nc():
    if _compiled[0] is None:
        _compiled[0] = _build()
    return _compiled[0]


# ================================ host prep =================================
def _slot_cols(slots):
    """featsT column base for each slot (conv psum-tile layout)."""
    s = np.asarray(slots)
    h = (s >= HALF).astype(np.int64)
    sl = s - h * HALF
    t = sl // 16
    i = sl % 16
    return 1024 * t + 512 * h + 32 * i


def _wrap16(a):
    """[N] -> [128, N//16] wrapped-16 index layout (replicated x8)."""
    n = a.shape[0]
    w = a.reshape(n // 16, 16).T
    return np.tile(w, (8, 1))


def _host_prep(pillars, coors_batch, npoints_per_pillar, batched_image_map,
               image_shapes, proj_mats, conv_w, bn_gamma, bn_beta, bn_mean,
               bn_var):
    from concourse import mybir
    np_bf16 = mybir.dt.np(mybir.dt.bfloat16)

    pillars = np.asarray(pillars, np.float32)
    coors = np.asarray(coors_batch, np.int64)
    npoints = np.asarray(npoints_per_pillar, np.int64)
    img = np.asarray(batched_image_map, np.float32)
    ish = np.asarray(image_shapes, np.int64)
    projm = np.asarray(proj_mats, np.float32)
    conv_w = np.asarray(conv_w, np.float32)
    bn_gamma = np.asarray(bn_gamma, np.float32)
    bn_beta = np.asarray(bn_beta, np.float32)
    bn_mean = np.asarray(bn_mean, np.float32)
    bn_var = np.asarray(bn_var, np.float32)

    xyz = pillars[:, :, :3]
    npf = npoints.astype(np.float32)
    mean = xyz.sum(axis=1) / npf[:, None]
    offset = xyz - mean[:, None, :]
    mask = np.arange(N_PTS)[None, :] < npoints[:, None]
    cx = coors[:, 1].astype(np.float32) * VX + X_OFF
    cy = coors[:, 2].astype(np.float32) * VY + Y_OFF
    x_off = pillars[:, :, 0] - cx[:, None]
    y_off = pillars[:, :, 1] - cy[:, None]
    feats9 = np.stack([x_off, y_off, pillars[:, :, 2], pillars[:, :, 3],
                       offset[:, :, 0], offset[:, :, 1], offset[:, :, 2],
                       x_off, y_off], axis=-1)
    feats9 = feats9 * mask[:, :, None].astype(np.float32)

    # top-K closest valid points (stable ties like lax.top_k)
    dists = np.where(mask, (offset * offset).sum(-1), np.float32(1e6))
    tidx = np.argsort(dists, axis=1, kind="stable")[:, :K_TOP]
    tmask = np.take_along_axis(mask, tidx, axis=1)
    txyz = np.take_along_axis(xyz, tidx[..., None], axis=1)

    b_idx = coors[:, 0]
    Mb = projm[b_idx]                                    # (P,3,4)
    hom = np.concatenate([txyz, np.ones_like(txyz[..., :1])], axis=-1)
    proj = np.einsum("pij,pkj->pki", Mb, hom).astype(np.float32)
    u = proj[..., 0] / proj[..., 2]
    v = proj[..., 1] / proj[..., 2]
    shf = ish[b_idx].astype(np.float32)                  # (P,2) H_img,W_img
    u_s = u * (np.float32(W_IMG) / shf[:, 1:2])
    v_s = v * (np.float32(H_IMG) / shf[:, 0:1])
    inside = (tmask & (u_s >= 0) & (u_s <= W_IMG - 1)
              & (v_s >= 0) & (v_s <= H_IMG - 1))
    u_c = np.clip(u_s, np.float32(0.0), np.float32(W_IMG - 1))
    v_c = np.clip(v_s, np.float32(0.0), np.float32(H_IMG - 1))
    x0 = np.floor(u_c).astype(np.int32)
    y0 = np.floor(v_c).astype(np.int32)
    y1 = np.minimum(y0 + 1, H_IMG - 1)
    wx = u_c - x0.astype(np.float32)
    wy = v_c - y0.astype(np.float32)
    w_in = inside.astype(np.float64)
    cnt = np.maximum(w_in.sum(axis=1), 1.0)
    wk = w_in / cnt[:, None]                             # (P,K)
    wx64, wy64 = wx.astype(np.float64), wy.astype(np.float64)
    # per (point, y-row h, x-half): folded bilinear weights
    cw = (np.stack([(1 - wx64) * (1 - wy64), wx64 * (1 - wy64),
                    (1 - wx64) * wy64, wx64 * wy64], axis=-1)
          * wk[..., None]).astype(np.float32)            # (P,K,4): 00,01,10,11
    rows_pair = np.stack([y0 * W_IMG + x0, y1 * W_IMG + x0], -1)  # (P,K,2)

    # folded conv+BN
    scale = bn_gamma / np.sqrt(bn_var + BN_EPS)
    Wp = conv_w * scale[:, None]                         # (64,9)
    bp = bn_beta - bn_mean * scale
    lhsT = np.zeros((10, 256), np.float32)
    lhsT[0:9, 0:64] = Wp.T
    lhsT[9, 0:64] = bp
    lhsT[0:9, 192:256] = Wp.T
    lhsT[9, 192:256] = bp
    lhsT = lhsT.astype(np_bf16)

    xc = coors[:, 1].astype(np.int64)
    yc = coors[:, 2].astype(np.int64)

    in_maps = []
    for core in range(N_CORES):
        b, r = divmod(core, N_REG)
        sel = np.where((b_idx == b) & (yc >= ROWS_REG * r)
                       & (yc < ROWS_REG * (r + 1)))[0]
        lin = (yc[sel] - ROWS_REG * r) * X_L + xc[sel]
        order = np.argsort(lin, kind="stable")
        sel = sel[order]
        lin = lin[order]
        n_p = len(sel)
        assert n_p <= PMAX, f"core {core}: {n_p} pillars > PMAX {PMAX}"

        # window-packed slot assignment: window c gets slots
        # [WSLOTS*c, WSLOTS*(c+1)); pads keep zero V rows and target the
        # window's private pad row WROWS.
        win = lin // WROWS                               # 0..3, sorted
        rel = (lin - win * WROWS).astype(np.int16)
        slot = np.empty(n_p, np.int64)
        widx = np.full(PMAX, WROWS, np.int16)
        for c in range(NWIN):
            m = np.nonzero(win == c)[0]
            assert len(m) <= WSLOTS, \
                f"core {core} window {c}: {len(m)} pillars > {WSLOTS}"
            slot[m] = WSLOTS * c + np.arange(len(m))
            widx[WSLOTS * c:WSLOTS * c + len(m)] = rel[m]

        ftT = np.zeros((10, FT_COLS), np_bf16)
        colb = _slot_cols(slot)
        cols = (colb[:, None] + np.arange(N_PTS)[None, :]).ravel()
        ftT[9, cols] = 1.0
        ftT[0:9].reshape(9, FT_COLS)[:, cols] = (
            feats9[sel].transpose(2, 0, 1).reshape(9, n_p * N_PTS)
            .astype(np_bf16))

        # gather indices (pads fetch row 0, weight 0) + folded weights
        gi = np.zeros((NG, PMAX), np.int16)
        gwv = np.zeros((NG, PB * 2, 128), np.float32)
        for k in range(K_TOP):
            for hf in range(2):
                g = k * 2 + hf
                gi[g, slot] = rows_pair[sel, k, hf].astype(np.int16)
                wlo = np.zeros(PMAX, np.float32)
                whi = np.zeros(PMAX, np.float32)
                wlo[slot] = cw[sel, k, 2 * hf]        # x0 cell
                whi[slot] = cw[sel, k, 2 * hf + 1]    # x1 cell
                gwv[g, 0::2, :] = wlo.reshape(PB, 128)
                gwv[g, 1::2, :] = whi.reshape(PB, 128)
        gidx = _wrap16(gi.reshape(NG * PMAX)).reshape(
            128, NG, PMAX // 16).reshape(128, NG * (PMAX // 16))
        gw2 = (gwv.reshape(NG * PB * 2, 128).T           # (128, NG*PB*2)
               .astype(np_bf16))

        imgt = np.zeros((HW + 2, 2 * C), np_bf16)
        imgt[:HW, :C] = (img[b].transpose(1, 2, 0).reshape(HW, C)
                         .astype(np_bf16))

        in_maps.append({
            "featsT": ftT,
            "lhsT": lhsT,
            "img": imgt,
            "gidx": np.ascontiguousarray(gidx),
            "gw2": np.ascontiguousarray(gw2),
            "widx": np.ascontiguousarray(_wrap16(widx)),
        })
    return in_maps


def _assemble(results):
    canvas_l = np.zeros((B, C, Y_L, X_L), np.float32)
    canvas_i = np.zeros((B, C, Y_L, X_L), np.float32)
    for core in range(N_CORES):
        b, r = divmod(core, N_REG)
        ysl = slice(ROWS_REG * r, ROWS_REG * (r + 1))
        arr = (results[core]["out_li"].reshape(NWIN, ROWS_PAD, 128)[:, :WROWS]
               .reshape(CELLS, 128).astype(np.float32))
        canvas_l[b, :, ysl, :] = arr[:, 0:C].T.reshape(C, ROWS_REG, X_L)
        canvas_i[b, :, ysl, :] = arr[:, C:2 * C].T.reshape(C, ROWS_REG, X_L)
    return canvas_l, canvas_i


def kernel(**inputs):
    from concourse.bass_utils import run_bass_kernel_spmd

    nc = _get_nc()
    in_maps = _host_prep(**inputs)
    res = run_bass_kernel_spmd(nc, in_maps, list(range(N_CORES)))
    return _assemble(res.results)
